# revision 1
# baseline (speedup 1.0000x reference)
"""Trainium2 Bass kernel for nn_MultiHeadAttention_89429809037632.

Linear attention (softplus feature map) with padding masks:
    q = query @ Wq.T ; k = key @ Wk.T ; v = key @ Wv.T   (per-head split)
    pq = softplus(q) ; pk = softplus(k) * keep(key_mask)
    kv = pk^T v (per head, plus a fused ones-column giving sum(pk))
    out = (pq @ kv) / (pq @ sum(pk)) * keep(query_mask)

Sharding across 8 NeuronCores: data-parallel over N=4 batches x
tensor-parallel over 2 head-groups (8 heads x 128 dims = 1024 output
dims each). Host transposes activations/weights so the contraction
dim (D) lands on the SBUF partition axis; each core runs an identical
SPMD program on its shard, outputs are concatenated on host.

Per-core program (Tile framework), default precision fp16 inputs with
fp32 PSUM accumulation (measured 4.4e-4 scale-relative absmax; set
PREC="f32r" for a 1.6e-4 variant ~17% slower):
  Phase A: for each 128-key chunk: project K,V (full-rate matmuls,
    stationary = key^T d-chunk serving 4 matmuls), softplus+mask -> pk,
    copy V into a [v | 1] block layout, then 8 per-head matmuls
    accumulate kv_aug (128x129 per head) in PSUM across all 32 chunks
    (one start=True per PSUM bank — start clears has_written bank-wide).
  Phase B: for each 512-query chunk x head: project Q, softplus -> pq,
    then per 128-query subchunk one matmul against kv_aug gives
    [num | den]; epilogue computes num * (keep/den) on DVE into a
    per-chunk staging tile shipped as one 512-run DMA.
  Matmul emission is software-pipelined (kv/num matmuls trail their
  producer chunk by one step) and weight/activation DMAs split across
  the sync and gpsimd DGEs to keep descriptor generation off the
  critical path.
"""

import json
import os
import sys
import types

import numpy as np

for _p in ("/opt/trn_rl_repo",):
    if _p not in sys.path and os.path.isdir(_p):
        sys.path.insert(0, _p)

# ``run_bass_kernel_spmd(trace=True)`` imports antenv.axon_hooks, which not
# every image ships. Provide a stub so the import never crashes (returning
# None simply disables NTFF tracing).
try:
    import antenv.axon_hooks  # noqa: F401
except Exception:
    try:
        import antenv

        _m = types.ModuleType("antenv.axon_hooks")
        _HOOK = [None]

        def _get_hook():
            if _HOOK[0] is None:
                try:
                    from trn_agent_boot.trn_boot import _ntff_profile_via_ctypes

                    _HOOK[0] = _ntff_profile_via_ctypes("/opt/axon/libaxon_pjrt.so")
                except Exception:
                    _HOOK[0] = None
            return _HOOK[0]

        _m.get_axon_ntff_profile_hook = _get_hook
        _m.set_axon_ntff_profile_hook = lambda h: _HOOK.__setitem__(0, h)
        sys.modules["antenv.axon_hooks"] = _m
        antenv.axon_hooks = _m
    except Exception:
        pass

import concourse.bass as bass
import concourse.bass_utils as bu
import concourse.mybir as mybir
import concourse.tile as tile

# ---------------------------------------------------------------------------
# Shim 1: this container's walrus accepts only ONE sync-wait per instruction
# ("Too many sync wait commands"); Tile attaches several. Rewrite the BIR
# JSON so excess waits ride on same-engine NoOps immediately before the
# instruction (engine streams are in-order, so this is equivalent).
# Shim 2: upload_artifacts wants a cloud bucket; keep artifacts local.
# ---------------------------------------------------------------------------
_MAX_WAITS = 1


def _split_multi_waits(bir_bytes: bytes) -> bytes:
    d = json.loads(bir_bytes)
    ctr = 0
    changed = False
    for fn in d.get("functions", []):
        for bb in fn.get("blocks", []):
            out = []
            for inst in bb.get("instructions", []):
                si = inst.get("sync_info")
                waits = (si or {}).get("on_wait") or []
                if len(waits) > _MAX_WAITS:
                    changed = True
                    idx = 0
                    while len(waits) - idx > _MAX_WAITS:
                        chunk = waits[idx : idx + _MAX_WAITS]
                        idx += _MAX_WAITS
                        ctr += 1
                        nop = {
                            "engine": inst["engine"],
                            "ins": [],
                            "outs": [],
                            "name": f"I-wsplit-{ctr}",
                            "opcode": "NoOp",
                            "sync_info": {"on_update": [], "on_wait": chunk},
                        }
                        if "debug" in inst:
                            nop["debug"] = inst["debug"]
                        out.append(nop)
                    si["on_wait"] = waits[idx:]
                out.append(inst)
            bb["instructions"] = out
    return json.dumps(d).encode() if changed else bir_bytes


if not getattr(bass.Bass, "_wait_split_shim", False):
    _orig_to_json = bass.Bass.to_json_bytes

    def _to_json_bytes(self) -> bytes:
        return _split_multi_waits(_orig_to_json(self))

    bass.Bass.to_json_bytes = _to_json_bytes
    bass.Bass._wait_split_shim = True
    bu.upload_artifacts = lambda tmpdir: tmpdir

# ---------------------------------------------------------------------------
# Problem shapes (hardcoded per contract)
# ---------------------------------------------------------------------------
N, L, D = 4, 4096, 2048  # batches, seq len (q and k), model dim
H, P = 16, 128  # heads, head dim
NCORES = 8
HL = H // 2  # heads per core (head-group of 8)
OW = HL * P  # per-core projected width (1024)
DC = D // P  # 16 contraction chunks
LC_A = L // P  # 32 key chunks (phase A)
LC_B = L // 512  # 8 query chunks of 512 (phase B)

F32 = mybir.dt.float32
F32R = mybir.dt.float32r
# The ACT tables in this walrus build ship no softplus; synthesize the
# numerically stable form softplus(x) = max(x,0) + ln(1 + exp(-|x|)) from
# set 6 ("natural_log_exp_and_others": abs/exp/ln in one resident table).
ABS = mybir.ActivationFunctionType.Abs
EXP = mybir.ActivationFunctionType.Exp
LN = mybir.ActivationFunctionType.Ln
MUL = mybir.AluOpType.mult
MAX = mybir.AluOpType.max
ADD = mybir.AluOpType.add

# kv_aug per-head column offsets inside the 3-bank PSUM accumulator:
# 3 heads per 2 KiB bank (129 fp32 columns each, none crossing a bank edge).
_KV_BASE = [(h // 3) * 512 + (h % 3) * 129 for h in range(HL)]

TRACE = False  # set True (e.g. from test.py) to capture NTFF profile
LAST_EXEC_TIME_NS = None

# Precision mode for all matmuls:
#   "f32r": projections in float32r (full rate, ~1.6e-4 end rel err),
#           attention stage fp32 (1/4-rate small matmuls, widened f32r num).
#   "fp16": everything fp16 inputs + fp32 PSUM accumulation (full rate,
#           ~11-bit input rounding => ~5e-4 end rel err, fastest).
#   "bf16": like fp16 but 8-bit mantissa (~2e-3 end rel err).
PREC = "fp16"

_CACHED_NC = {}


def _build_nc() -> bass.Bass:
    from contextlib import ExitStack

    lp = {"fp16": mybir.dt.float16, "bf16": mybir.dt.bfloat16}.get(PREC)
    IN_DT = F32R if lp is None else lp  # projections (dram + sbuf operands)
    att_dt = F32 if lp is None else lp  # kv-stage operands (pk, v_aug)
    # num-stage matmul: fp16/bf16 run full-rate at any width; the f32r path
    # widens to 258 moving columns (>=256 streams at 1 cyc/row vs fp32's 4)
    # — the upper 129 columns are discarded junk.
    num_dt = F32R if lp is None else lp
    NW = 258 if lp is None else 129
    KV_W = HL * 129 + (129 if lp is None else 0)  # pad so h=7 reads 258 cols

    nc = bass.Bass()
    qT = nc.dram_tensor("qT", (D, L), IN_DT, kind="ExternalInput")
    kT = nc.dram_tensor("kT", (D, L), IN_DT, kind="ExternalInput")
    wq = nc.dram_tensor("wq", (D, OW), IN_DT, kind="ExternalInput")
    wk = nc.dram_tensor("wk", (D, OW), IN_DT, kind="ExternalInput")
    wv = nc.dram_tensor("wv", (D, OW), IN_DT, kind="ExternalInput")
    qm = nc.dram_tensor("qm", (P, LC_A), F32, kind="ExternalInput")
    km = nc.dram_tensor("km", (P, LC_A), F32, kind="ExternalInput")
    out = nc.dram_tensor("out", (L, OW), F32, kind="ExternalOutput")

    with tile.TileContext(nc) as tc, ExitStack() as outer:
        misc = outer.enter_context(tc.tile_pool(name="misc", bufs=1))
        kvpool = outer.enter_context(tc.tile_pool(name="kvsb", bufs=1))
        early_wq = lp is not None  # half-size weights fit beside phase A
        if early_wq:
            wqp_early = outer.enter_context(tc.tile_pool(name="wqe", bufs=1))
        qm_sb = misc.tile([P, LC_A], F32)
        km_sb = misc.tile([P, LC_A], F32)
        nc.sync.dma_start(qm_sb[:], qm[:])
        nc.sync.dma_start(km_sb[:], km[:])
        kv_sb = kvpool.tile([P, KV_W], num_dt)

        # ------ Phase A: K/V projection + kv accumulation ------------------
        # One pass, all 8 heads. Each kt d-chunk tile serves 4 consecutive
        # matmuls (K and V, two 512-wide o-halves each) — consecutive
        # matmuls sharing the stationary amortize the f32r internal weight
        # load (measured: splitting to 2 MMs/stationary costs ~15% PE time).
        esA = ExitStack()
        wkvp = esA.enter_context(tc.tile_pool(name="wkv", bufs=1))
        ktp = esA.enter_context(tc.tile_pool(name="kt", bufs=2))
        pkp = esA.enter_context(tc.tile_pool(name="pk", bufs=3))
        vap = esA.enter_context(tc.tile_pool(name="vaug", bufs=3))
        pps = esA.enter_context(tc.tile_pool(name="projps", bufs=5, space="PSUM"))
        kvps = esA.enter_context(tc.tile_pool(name="kvps", bufs=1, space="PSUM"))
        kv_ps = kvps.tile([P, 1536], F32)

        def load_kt_chunk(c):
            tiles = [
                ktp.tile([P, P], IN_DT, tag=f"kt{dc}", name=f"kt{dc}")
                for dc in range(DC)
            ]
            for dc in range(DC):
                nc.sync.dma_start(
                    tiles[dc][:], kT[dc * P : (dc + 1) * P, c * P : (c + 1) * P]
                )
            return tiles

        # Chunk-0 kt DMAs go FIRST so the opening matmuls wait on ~1 MB,
        # not the 16 MB weight preload queued behind them.
        kt_c0 = load_kt_chunk(0)

        wk_sb = [
            wkvp.tile([P, OW], IN_DT, tag=f"wk{dc}", name=f"wk{dc}")
            for dc in range(DC)
        ]
        wv_sb = [
            wkvp.tile([P, OW], IN_DT, tag=f"wv{dc}", name=f"wv{dc}")
            for dc in range(DC)
        ]
        for dc in range(DC):
            nc.gpsimd.dma_start(wk_sb[dc][:], wk[dc * P : (dc + 1) * P, :])
            nc.gpsimd.dma_start(wv_sb[dc][:], wv[dc * P : (dc + 1) * P, :])
        if early_wq:
            wq_sb = [
                wqp_early.tile([P, OW], IN_DT, tag=f"wq{dc}", name=f"wq{dc}")
                for dc in range(DC)
            ]
            for dc in range(DC):
                nc.gpsimd.dma_start(wq_sb[dc][:], wq[dc * P : (dc + 1) * P, :])

        bank_start = {}

        def emit_kv_mms(c, pk_sb, va_sb):
            for h in range(HL):
                bank_first = h % 3 == 0
                mm = nc.tensor.matmul(
                    kv_ps[:, _KV_BASE[h] : _KV_BASE[h] + 129],
                    pk_sb[:, h * P : (h + 1) * P],
                    va_sb[:, h * 129 : (h + 1) * 129],
                    start=(c == 0 and bank_first),
                    stop=(c == LC_A - 1),
                    skip_group_check=True,
                )
                if c == 0:
                    # start=True clears has_written for the whole PSUM bank;
                    # siblings must come after their bank's clear.
                    if bank_first:
                        bank_start[h // 3] = mm
                    else:
                        tile.add_dep_helper(
                            mm.ins,
                            bank_start[h // 3].ins,
                            reason="kv bank has_written clear order",
                        )

        # kv matmuls for chunk c are emitted after chunk c+1's projection
        # matmuls: their pk operand is only ready ~3us after chunk c's last
        # projection, so this keeps PE fed meanwhile.
        pending = None
        for c in range(LC_A):
            kt_sb = kt_c0 if c == 0 else load_kt_chunk(c)
            kp0 = pps.tile([P, 512], F32, tag="proj", name="kp0")
            kp1 = pps.tile([P, 512], F32, tag="proj", name="kp1")
            vp0 = pps.tile([P, 512], F32, tag="proj", name="vp0")
            vp1 = pps.tile([P, 512], F32, tag="proj", name="vp1")
            for dc in range(DC):
                lhsT = kt_sb[dc][:]
                st = dict(start=(dc == 0), stop=(dc == DC - 1))
                nc.tensor.matmul(kp0[:], lhsT, wk_sb[dc][:, 0:512], **st)
                nc.tensor.matmul(kp1[:], lhsT, wk_sb[dc][:, 512:1024], **st)
                nc.tensor.matmul(vp0[:], lhsT, wv_sb[dc][:, 0:512], **st)
                nc.tensor.matmul(vp1[:], lhsT, wv_sb[dc][:, 512:1024], **st)

            if pending is not None:
                emit_kv_mms(*pending)

            pk_sb = pkp.tile([P, OW], att_dt, tag="pk", name="pk")
            for half, kp in ((0, kp0), (1, kp1)):
                sa = pkp.tile([P, 512], F32, tag="sa", name="sa")
                sb = pkp.tile([P, 512], F32, tag="sb", name="sb")
                nc.scalar.activation(sa[:], kp[:], ABS)
                nc.scalar.activation(sb[:], sa[:], EXP, scale=-1.0)
                nc.scalar.activation(sa[:], sb[:], LN, bias=1.0)
                nc.vector.scalar_tensor_tensor(
                    pk_sb[:, half * 512 : (half + 1) * 512],
                    kp[:],
                    0.0,
                    sa[:],
                    MAX,
                    ADD,
                )
            nc.vector.tensor_scalar_mul(pk_sb[:], pk_sb[:], km_sb[:, c : c + 1])

            va_sb = vap.tile([P, HL * 129], att_dt, tag="vaug", name="va")
            nc.vector.memset(
                va_sb[:].rearrange("p (h x) -> p h x", x=129)[:, :, 128:129], 1.0
            )
            for h in range(HL):
                src = vp0 if h < 4 else vp1
                off = (h % 4) * P
                nc.vector.tensor_copy(
                    va_sb[:, h * 129 : h * 129 + P], src[:, off : off + P]
                )
            pending = (c, pk_sb, va_sb)
        emit_kv_mms(*pending)

        for h in range(HL):
            nc.vector.tensor_copy(
                kv_sb[:, h * 129 : (h + 1) * 129],
                kv_ps[:, _KV_BASE[h] : _KV_BASE[h] + 129],
            )
        if KV_W > HL * 129:
            # f32r-typed zero pad (junk columns read by head 7's widened MM);
            # written by a DVE op so the FP32R-rounding verifier is satisfied.
            nc.vector.tensor_scalar_mul(
                kv_sb[:, HL * 129 : KV_W], kv_sb[:, 0:129], 0.0
            )
        esA.close()

        # ------ Phase B: Q projection + attention epilogue -----------------
        with (
            ExitStack() as esB,
            tc.tile_pool(name="qt", bufs=2) as qtp,
            tc.tile_pool(name="pq", bufs=3) as pqp,
            tc.tile_pool(name="sc", bufs=4) as scp,
            tc.tile_pool(name="st", bufs=3) as stp,
            tc.tile_pool(name="qpps", bufs=4, space="PSUM") as qpps,
            tc.tile_pool(name="nmps", bufs=4, space="PSUM") as nmps,
        ):
            if not early_wq:
                wqp = esB.enter_context(tc.tile_pool(name="wq", bufs=1))
                wq_sb = [
                    wqp.tile([P, OW], IN_DT, tag=f"wq{dc}", name=f"wq{dc}")
                    for dc in range(DC)
                ]
                for dc in range(DC):
                    nc.gpsimd.dma_start(
                        wq_sb[dc][:], wq[dc * P : (dc + 1) * P, :]
                    )

            st_tiles = {}

            def emit_num(lc, h, pq_sb):
                # results stage into st (partition=l%128, cols j*OW+o) and
                # ship as ONE 512-run DMA per lc — per-(h,j) output DMAs cost
                # ~0.6us of sync-sequencer descriptor generation each.
                if h == 0:
                    st_tiles[lc] = stp.tile([P, 4 * OW], F32, tag="st", name="st")
                st = st_tiles[lc]
                for j in range(4):
                    nm = nmps.tile([P, NW], F32, tag="nm", name="nm")
                    nc.tensor.matmul(
                        nm[:],
                        pq_sb[:, j * P : (j + 1) * P],
                        kv_sb[:, h * 129 : h * 129 + NW],
                        start=True,
                        stop=True,
                    )
                    sc = scp.tile([P, 1], F32, tag="sc", name="sc")
                    nc.vector.reciprocal(sc[:], nm[:, 128:129])
                    col = lc * 4 + j
                    nc.vector.tensor_scalar(
                        st[:, j * OW + h * P : j * OW + (h + 1) * P],
                        nm[:, 0:P],
                        sc[:, 0:1],
                        qm_sb[:, col : col + 1],
                        MUL,
                        MUL,
                    )
                if h == HL - 1:
                    nc.sync.dma_start(
                        out[lc * 512 : (lc + 1) * 512, :].rearrange(
                            "(j p) o -> p j o", p=P
                        ),
                        st[:].rearrange("p (j o) -> p j o", o=OW),
                    )
                    del st_tiles[lc]

            # num matmuls for step (lc,h) are emitted after step (lc,h)+1's
            # projection matmuls (pq is ~2.5us of ACT/DVE behind qp).
            pending = None
            for lc in range(LC_B):
                qt_sb = [
                    qtp.tile([P, 512], IN_DT, tag=f"qt{dc}", name=f"qt{dc}")
                    for dc in range(DC)
                ]
                for dc in range(DC):
                    nc.gpsimd.dma_start(
                        qt_sb[dc][:],
                        qT[dc * P : (dc + 1) * P, lc * 512 : (lc + 1) * 512],
                    )
                for h in range(HL):
                    qp = qpps.tile([P, 512], F32, tag="qp", name="qp")
                    for dc in range(DC):
                        nc.tensor.matmul(
                            qp[:],
                            wq_sb[dc][:, h * P : (h + 1) * P],
                            qt_sb[dc][:],
                            start=(dc == 0),
                            stop=(dc == DC - 1),
                        )
                    if pending is not None:
                        emit_num(*pending)
                    pq_sb = pqp.tile([P, 512], num_dt, tag="pq", name="pq")
                    sa = pqp.tile([P, 512], F32, tag="sqa", name="sqa")
                    sb = pqp.tile([P, 512], F32, tag="sqb", name="sqb")
                    nc.scalar.activation(sa[:], qp[:], ABS)
                    nc.scalar.activation(sb[:], sa[:], EXP, scale=-1.0)
                    nc.scalar.activation(sa[:], sb[:], LN, bias=1.0)
                    nc.vector.scalar_tensor_tensor(
                        pq_sb[:], qp[:], 0.0, sa[:], MAX, ADD
                    )
                    pending = (lc, h, pq_sb)
            emit_num(*pending)
    return nc


def _get_nc() -> bass.Bass:
    if PREC not in _CACHED_NC:
        _CACHED_NC[PREC] = _build_nc()
    return _CACHED_NC[PREC]


def kernel(query, key, Wq, Wk, Wv, query_padding_mask, key_padding_mask):
    global LAST_EXEC_TIME_NS
    query = np.asarray(query, dtype=np.float32)
    key = np.asarray(key, dtype=np.float32)
    Wq = np.asarray(Wq, dtype=np.float32)
    Wk = np.asarray(Wk, dtype=np.float32)
    Wv = np.asarray(Wv, dtype=np.float32)
    qmask = np.asarray(query_padding_mask)
    kmask = np.asarray(key_padding_mask)

    nc = _get_nc()

    in_dt = np.float32
    if PREC == "fp16":
        in_dt = np.float16
    elif PREC == "bf16":
        import ml_dtypes

        in_dt = ml_dtypes.bfloat16

    in_maps = []
    for c in range(NCORES):
        n, g = c // 2, c % 2
        sl = slice(g * OW, (g + 1) * OW)
        qkeep = (~qmask[n]).astype(np.float32).reshape(LC_A, P).T
        kkeep = (~kmask[n]).astype(np.float32).reshape(LC_A, P).T
        in_maps.append(
            {
                "qT": np.ascontiguousarray(query[n].T.astype(in_dt)),
                "kT": np.ascontiguousarray(key[n].T.astype(in_dt)),
                "wq": np.ascontiguousarray(Wq[sl, :].T.astype(in_dt)),
                "wk": np.ascontiguousarray(Wk[sl, :].T.astype(in_dt)),
                "wv": np.ascontiguousarray(Wv[sl, :].T.astype(in_dt)),
                "qm": np.ascontiguousarray(qkeep),
                "km": np.ascontiguousarray(kkeep),
            }
        )

    res = bu.run_bass_kernel_spmd(
        nc, in_maps, core_ids=list(range(NCORES)), trace=TRACE
    )
    LAST_EXEC_TIME_NS = res.exec_time_ns

    full = np.empty((N, L, D), dtype=np.float32)
    for c in range(NCORES):
        n, g = c // 2, c % 2
        full[n, :, g * OW : (g + 1) * OW] = res.results[c]["out"]
    return full



# revision 3
# speedup vs baseline: 1.0995x; 1.0995x over previous
"""Trainium2 Bass kernel for nn_MultiHeadAttention_89429809037632.

Linear attention (softplus feature map) with padding masks:
    q = query @ Wq.T ; k = key @ Wk.T ; v = key @ Wv.T   (per-head split)
    pq = softplus(q) ; pk = softplus(k) * keep(key_mask)
    kv = pk^T v (per head, plus a fused ones-column giving sum(pk))
    out = (pq @ kv) / (pq @ sum(pk)) * keep(query_mask)

Sharding across 8 NeuronCores: data-parallel over N=4 batches x
tensor-parallel over 2 head-groups (8 heads x 128 dims = 1024 output
dims each). Host transposes activations/weights so the contraction
dim (D) lands on the SBUF partition axis; each core runs an identical
SPMD program on its shard, outputs are concatenated on host.

Padding compaction: ~10% of keys/queries are padded (masked to zero
contribution / zero output). The host gathers the unpadded rows,
pads to a common 128-multiple across batches (compiled shapes depend
only on the rounded counts, cached), and scatters the output back.
Removed keys contribute exactly 0 (pk=0) so numerics are unchanged;
this cuts ~9% of all three projection GEMMs (PE issue time is the
bottleneck at 94% tensor-engine occupancy).

Per-core program (Tile framework), fp16 inputs with fp32 PSUM
accumulation (measured ~4.4e-4 scale-relative absmax):
  Warmup: ~96 dependency-free dummy matmuls issued at t=0 keep the PE
    busy while the first DMAs land, flipping the HAM clock gate from
    1.2GHz to 2.4GHz before real work starts.
  Phase A: for each 128-key chunk: project K,V (full-rate matmuls,
    stationary = key^T d-chunk serving 4 matmuls), softplus+mask -> pk,
    copy V into a [v | 1] block layout, then 8 per-head matmuls
    accumulate kv_aug (128x129 per head) in PSUM across all chunks.
  Phase B: for each query chunk (512-query chunks + one remainder) x
    head: project Q, softplus -> pq, then per 128-query subchunk one
    matmul against kv_aug gives [num | den]; epilogue computes
    num * (keep/den) on DVE into a per-chunk staging tile shipped as
    one chunk-sized DMA.
  kT/qT are host-packed so each chunk tile loads with ONE dma_start of
  4-16KB contiguous per-partition runs. DMA descriptor generation is
  spread over three rings (kt+out on sync, wk+wq+qt on gpsimd, wv on
  scalar) so the opening weight stream isn't serialized behind one
  sequencer. Matmul emission is software-pipelined (kv/num matmuls
  trail their producer chunk by one step).
"""

import json
import os
import sys
import types

import numpy as np

for _p in ("/opt/trn_rl_repo",):
    if _p not in sys.path and os.path.isdir(_p):
        sys.path.insert(0, _p)

# ``run_bass_kernel_spmd(trace=True)`` imports antenv.axon_hooks, which not
# every image ships. Provide a stub so the import never crashes (returning
# None simply disables NTFF tracing).
try:
    import antenv.axon_hooks  # noqa: F401
except Exception:
    try:
        import antenv

        _m = types.ModuleType("antenv.axon_hooks")
        _HOOK = [None]

        def _get_hook():
            if _HOOK[0] is None:
                try:
                    from trn_agent_boot.trn_boot import _ntff_profile_via_ctypes

                    _HOOK[0] = _ntff_profile_via_ctypes("/opt/axon/libaxon_pjrt.so")
                except Exception:
                    _HOOK[0] = None
            return _HOOK[0]

        _m.get_axon_ntff_profile_hook = _get_hook
        _m.set_axon_ntff_profile_hook = lambda h: _HOOK.__setitem__(0, h)
        sys.modules["antenv.axon_hooks"] = _m
        antenv.axon_hooks = _m
    except Exception:
        pass

import concourse.bass as bass
import concourse.bass_utils as bu
import concourse.mybir as mybir
import concourse.tile as tile

# ---------------------------------------------------------------------------
# Shim 1: this container's walrus accepts only ONE sync-wait per instruction
# ("Too many sync wait commands"); Tile attaches several. Rewrite the BIR
# JSON so excess waits ride on same-engine NoOps immediately before the
# instruction (engine streams are in-order, so this is equivalent).
# Shim 2: upload_artifacts wants a cloud bucket; keep artifacts local.
# ---------------------------------------------------------------------------
_MAX_WAITS = 1


def _split_multi_waits(bir_bytes: bytes) -> bytes:
    d = json.loads(bir_bytes)
    ctr = 0
    changed = False
    for fn in d.get("functions", []):
        for bb in fn.get("blocks", []):
            out = []
            for inst in bb.get("instructions", []):
                si = inst.get("sync_info")
                waits = (si or {}).get("on_wait") or []
                if len(waits) > _MAX_WAITS:
                    changed = True
                    idx = 0
                    while len(waits) - idx > _MAX_WAITS:
                        chunk = waits[idx : idx + _MAX_WAITS]
                        idx += _MAX_WAITS
                        ctr += 1
                        nop = {
                            "engine": inst["engine"],
                            "ins": [],
                            "outs": [],
                            "name": f"I-wsplit-{ctr}",
                            "opcode": "NoOp",
                            "sync_info": {"on_update": [], "on_wait": chunk},
                        }
                        if "debug" in inst:
                            nop["debug"] = inst["debug"]
                        out.append(nop)
                    si["on_wait"] = waits[idx:]
                out.append(inst)
            bb["instructions"] = out
    return json.dumps(d).encode() if changed else bir_bytes


if not getattr(bass.Bass, "_wait_split_shim", False):
    _orig_to_json = bass.Bass.to_json_bytes

    def _to_json_bytes(self) -> bytes:
        return _split_multi_waits(_orig_to_json(self))

    bass.Bass.to_json_bytes = _to_json_bytes
    bass.Bass._wait_split_shim = True
    bu.upload_artifacts = lambda tmpdir: tmpdir

# ---------------------------------------------------------------------------
# Problem shapes (hardcoded per contract)
# ---------------------------------------------------------------------------
N, L, D = 4, 4096, 2048  # batches, seq len (q and k), model dim
H, P = 16, 128  # heads, head dim
NCORES = 8
HL = H // 2  # heads per core (head-group of 8)
OW = HL * P  # per-core projected width (1024)
DC = D // P  # 16 contraction chunks

F32 = mybir.dt.float32
F32R = mybir.dt.float32r
# The ACT tables in this walrus build ship no softplus; synthesize the
# numerically stable form softplus(x) = max(x,0) + ln(1 + exp(-|x|)) from
# set 6 ("natural_log_exp_and_others": abs/exp/ln in one resident table).
ABS = mybir.ActivationFunctionType.Abs
EXP = mybir.ActivationFunctionType.Exp
LN = mybir.ActivationFunctionType.Ln
MUL = mybir.AluOpType.mult
MAX = mybir.AluOpType.max
ADD = mybir.AluOpType.add

# kv_aug per-head column offsets inside the 3-bank PSUM accumulator:
# 3 heads per 2 KiB bank (129 fp32 columns each, none crossing a bank edge).
_KV_BASE = [(h // 3) * 512 + (h % 3) * 129 for h in range(HL)]

TRACE = False  # set True (e.g. from test.py) to capture NTFF profile
LAST_EXEC_TIME_NS = None

# Precision mode for all matmuls:
#   "f32r": projections in float32r (full rate, ~1.6e-4 end rel err),
#           attention stage fp32 (1/4-rate small matmuls, widened f32r num).
#   "fp16": everything fp16 inputs + fp32 PSUM accumulation (full rate,
#           ~11-bit input rounding => ~5e-4 end rel err, fastest).
#   "bf16": like fp16 but 8-bit mantissa (~2e-3 end rel err).
PREC = "fp16"

N_WARM = 96  # dummy matmuls to warm the HAM clock gate while DMAs land

_CACHED_NC = {}


def _build_nc(kc: int, qchunks: tuple) -> bass.Bass:
    """Build the per-core program for kc 128-key chunks and the given
    query chunk widths (each a multiple of 128; 512 in steady state)."""
    from contextlib import ExitStack

    qtot = sum(qchunks)
    qcb = qtot // P  # query keep-mask columns

    lp = {"fp16": mybir.dt.float16, "bf16": mybir.dt.bfloat16}.get(PREC)
    IN_DT = F32R if lp is None else lp  # projections (dram + sbuf operands)
    att_dt = F32 if lp is None else lp  # kv-stage operands (pk, v_aug)
    # num-stage matmul: fp16/bf16 run full-rate at any width; the f32r path
    # widens to 258 moving columns (>=256 streams at 1 cyc/row vs fp32's 4)
    # — the upper 129 columns are discarded junk.
    num_dt = F32R if lp is None else lp
    NW = 258 if lp is None else 129
    KV_W = HL * 129 + (129 if lp is None else 0)  # pad so h=7 reads 258 cols

    nc = bass.Bass()
    # qT/kT are host-packed: column block c holds that chunk's activations
    # with layout [p, dc*W + j] = act[chunk_off + j, dc*128 + p], so a chunk
    # loads as ONE dma_start with a contiguous per-partition run and the
    # dc-slices come out as plain column slices.
    qT = nc.dram_tensor("qT", (P, DC * qtot), IN_DT, kind="ExternalInput")
    kT = nc.dram_tensor("kT", (P, kc * D), IN_DT, kind="ExternalInput")
    wq = nc.dram_tensor("wq", (D, OW), IN_DT, kind="ExternalInput")
    wk = nc.dram_tensor("wk", (D, OW), IN_DT, kind="ExternalInput")
    wv = nc.dram_tensor("wv", (D, OW), IN_DT, kind="ExternalInput")
    qm = nc.dram_tensor("qm", (P, qcb), F32, kind="ExternalInput")
    km = nc.dram_tensor("km", (P, kc), F32, kind="ExternalInput")
    out = nc.dram_tensor("out", (qtot, OW), F32, kind="ExternalOutput")

    with tile.TileContext(nc) as tc, ExitStack() as outer:
        # ------ PE warmup: the HAM clock gate starts at 1.2GHz and needs
        # ~3.4us of sustained activity to release to 2.4GHz. The opening
        # DMAs take ~10us to land; fill that window with dependency-free
        # matmuls so real work starts warm. Pool closes to free the bank.
        with (
            tc.tile_pool(name="warm", bufs=1) as warmp,
            tc.tile_pool(name="warmps", bufs=1, space="PSUM") as warmpsp,
        ):
            wt = warmp.tile([P, P], IN_DT, name="wt")
            nc.vector.memset(wt[:], 0.0)
            wps = warmpsp.tile([P, P], F32, name="wps")
            for _ in range(N_WARM):
                nc.tensor.matmul(wps[:], wt[:], wt[:], start=True, stop=True)

        misc = outer.enter_context(tc.tile_pool(name="misc", bufs=1))
        kvpool = outer.enter_context(tc.tile_pool(name="kvsb", bufs=1))
        # qt pool lives at OUTER scope so its SBUF region is disjoint from
        # the phase-A pools: the first chunks' qt DMAs (ring-ahead on the
        # gpsimd queue) then stream in during phase A with no anti-deps.
        qtp = outer.enter_context(tc.tile_pool(name="qt", bufs=2))
        wqp = outer.enter_context(tc.tile_pool(name="wqe", bufs=1))
        qm_sb = misc.tile([P, qcb], F32)
        km_sb = misc.tile([P, kc], F32)
        nc.scalar.dma_start(qm_sb[:], qm[:])
        nc.scalar.dma_start(km_sb[:], km[:])
        kv_sb = kvpool.tile([P, KV_W], num_dt)

        # ------ Phase A: K/V projection + kv accumulation ------------------
        # One pass, all 8 heads. Each kt d-chunk slice serves 4 consecutive
        # matmuls (K and V, two 512-wide o-halves each).
        esA = ExitStack()
        wkvp = esA.enter_context(tc.tile_pool(name="wkv", bufs=1))
        ktp = esA.enter_context(tc.tile_pool(name="kt", bufs=3))
        pkp = esA.enter_context(tc.tile_pool(name="pk", bufs=3))
        vap = esA.enter_context(tc.tile_pool(name="vaug", bufs=3))
        pps = esA.enter_context(tc.tile_pool(name="projps", bufs=5, space="PSUM"))
        kvps = esA.enter_context(tc.tile_pool(name="kvps", bufs=1, space="PSUM"))
        kv_ps = kvps.tile([P, 1536], F32)

        # kt chunk 0 FIRST on the sync ring so the opening matmuls wait on
        # 0.5 MB; weights stream concurrently on the gpsimd + scalar rings.
        kt0 = ktp.tile([P, D], IN_DT, tag="kt", name="kt0")
        nc.sync.dma_start(kt0[:], kT[:, 0:D])

        wk_sb = [
            wkvp.tile([P, OW], IN_DT, tag=f"wk{dc}", name=f"wk{dc}")
            for dc in range(DC)
        ]
        wv_sb = [
            wkvp.tile([P, OW], IN_DT, tag=f"wv{dc}", name=f"wv{dc}")
            for dc in range(DC)
        ]
        for dc in range(DC):
            nc.gpsimd.dma_start(wk_sb[dc][:], wk[dc * P : (dc + 1) * P, :])
            nc.scalar.dma_start(wv_sb[dc][:], wv[dc * P : (dc + 1) * P, :])
        wq_sb = [
            wqp.tile([P, OW], IN_DT, tag=f"wq{dc}", name=f"wq{dc}")
            for dc in range(DC)
        ]
        for dc in range(DC):
            nc.gpsimd.dma_start(wq_sb[dc][:], wq[dc * P : (dc + 1) * P, :])

        bank_start = {}

        def emit_kv_mms(c, pk_sb, va_sb):
            for h in range(HL):
                bank_first = h % 3 == 0
                mm = nc.tensor.matmul(
                    kv_ps[:, _KV_BASE[h] : _KV_BASE[h] + 129],
                    pk_sb[:, h * P : (h + 1) * P],
                    va_sb[:, h * 129 : (h + 1) * 129],
                    start=(c == 0 and bank_first),
                    stop=(c == kc - 1),
                    skip_group_check=True,
                )
                if c == 0:
                    # start=True clears has_written for the whole PSUM bank;
                    # siblings must come after their bank's clear.
                    if bank_first:
                        bank_start[h // 3] = mm
                    else:
                        tile.add_dep_helper(
                            mm.ins,
                            bank_start[h // 3].ins,
                            reason="kv bank has_written clear order",
                        )

        # kv matmuls for chunk c are emitted after chunk c+1's projection
        # matmuls: their pk operand is only ready ~3us after chunk c's last
        # projection, so this keeps PE fed meanwhile.
        pending = None
        for c in range(kc):
            if c == 0:
                kt_sb = kt0
            else:
                kt_sb = ktp.tile([P, D], IN_DT, tag="kt", name=f"kt{c}")
                nc.sync.dma_start(kt_sb[:], kT[:, c * D : (c + 1) * D])
            kp0 = pps.tile([P, 512], F32, tag="proj", name="kp0")
            kp1 = pps.tile([P, 512], F32, tag="proj", name="kp1")
            vp0 = pps.tile([P, 512], F32, tag="proj", name="vp0")
            vp1 = pps.tile([P, 512], F32, tag="proj", name="vp1")
            for dc in range(DC):
                lhsT = kt_sb[:, dc * P : (dc + 1) * P]
                st = dict(start=(dc == 0), stop=(dc == DC - 1))
                nc.tensor.matmul(kp0[:], lhsT, wk_sb[dc][:, 0:512], **st)
                nc.tensor.matmul(kp1[:], lhsT, wk_sb[dc][:, 512:1024], **st)
                nc.tensor.matmul(vp0[:], lhsT, wv_sb[dc][:, 0:512], **st)
                nc.tensor.matmul(vp1[:], lhsT, wv_sb[dc][:, 512:1024], **st)

            if pending is not None:
                emit_kv_mms(*pending)

            pk_sb = pkp.tile([P, OW], att_dt, tag="pk", name="pk")
            for half, kp in ((0, kp0), (1, kp1)):
                sa = pkp.tile([P, 512], F32, tag="sa", name="sa")
                sb = pkp.tile([P, 512], F32, tag="sb", name="sb")
                nc.scalar.activation(sa[:], kp[:], ABS)
                nc.scalar.activation(sb[:], sa[:], EXP, scale=-1.0)
                nc.scalar.activation(sa[:], sb[:], LN, bias=1.0)
                nc.vector.scalar_tensor_tensor(
                    pk_sb[:, half * 512 : (half + 1) * 512],
                    kp[:],
                    0.0,
                    sa[:],
                    MAX,
                    ADD,
                )
            nc.vector.tensor_scalar_mul(pk_sb[:], pk_sb[:], km_sb[:, c : c + 1])

            va_sb = vap.tile([P, HL * 129], att_dt, tag="vaug", name="va")
            nc.vector.memset(
                va_sb[:].rearrange("p (h x) -> p h x", x=129)[:, :, 128:129], 1.0
            )
            for h in range(HL):
                src = vp0 if h < 4 else vp1
                off = (h % 4) * P
                nc.vector.tensor_copy(
                    va_sb[:, h * 129 : h * 129 + P], src[:, off : off + P]
                )
            pending = (c, pk_sb, va_sb)
        emit_kv_mms(*pending)

        for h in range(HL):
            nc.vector.tensor_copy(
                kv_sb[:, h * 129 : (h + 1) * 129],
                kv_ps[:, _KV_BASE[h] : _KV_BASE[h] + 129],
            )
        if KV_W > HL * 129:
            # f32r-typed zero pad (junk columns read by head 7's widened MM);
            # written by a DVE op so the FP32R-rounding verifier is satisfied.
            nc.vector.tensor_scalar_mul(
                kv_sb[:, HL * 129 : KV_W], kv_sb[:, 0:129], 0.0
            )
        esA.close()

        # ------ Phase B: Q projection + attention epilogue -----------------
        with (
            tc.tile_pool(name="pq", bufs=3) as pqp,
            tc.tile_pool(name="sc", bufs=4) as scp,
            tc.tile_pool(name="st", bufs=3) as stp,
            tc.tile_pool(name="qpps", bufs=4, space="PSUM") as qpps,
            tc.tile_pool(name="nmps", bufs=4, space="PSUM") as nmps,
        ):
            st_tiles = {}

            def emit_num(ci, o, W, h, pq_sb):
                # results stage into st (partition=l%128, cols j*OW+o) and
                # ship as ONE multi-run DMA per chunk — per-(h,j) output
                # DMAs cost ~0.6us of descriptor generation each.
                nj = W // P
                if h == 0:
                    st_tiles[ci] = stp.tile([P, nj * OW], F32, tag="st", name="st")
                st = st_tiles[ci]
                for j in range(nj):
                    nm = nmps.tile([P, NW], F32, tag="nm", name="nm")
                    nc.tensor.matmul(
                        nm[:],
                        pq_sb[:, j * P : (j + 1) * P],
                        kv_sb[:, h * 129 : h * 129 + NW],
                        start=True,
                        stop=True,
                    )
                    sc = scp.tile([P, 1], F32, tag="sc", name="sc")
                    nc.vector.reciprocal(sc[:], nm[:, 128:129])
                    col = o // P + j
                    nc.vector.tensor_scalar(
                        st[:, j * OW + h * P : j * OW + (h + 1) * P],
                        nm[:, 0:P],
                        sc[:, 0:1],
                        qm_sb[:, col : col + 1],
                        MUL,
                        MUL,
                    )
                if h == HL - 1:
                    nc.sync.dma_start(
                        out[o : o + W, :].rearrange("(j p) o -> p j o", p=P),
                        st[:].rearrange("p (j o) -> p j o", o=OW),
                    )
                    del st_tiles[ci]

            # num matmuls for step (ci,h) are emitted after step (ci,h)+1's
            # projection matmuls (pq is ~2.5us of ACT/DVE behind qp).
            pending = None
            o = 0
            for ci, W in enumerate(qchunks):
                qt_sb = qtp.tile([P, DC * W], IN_DT, tag="qt", name=f"qt{ci}")
                nc.gpsimd.dma_start(
                    qt_sb[:], qT[:, DC * o : DC * (o + W)]
                )
                for h in range(HL):
                    qp = qpps.tile([P, W], F32, tag="qp", name="qp")
                    for dc in range(DC):
                        nc.tensor.matmul(
                            qp[:],
                            wq_sb[dc][:, h * P : (h + 1) * P],
                            qt_sb[:, dc * W : (dc + 1) * W],
                            start=(dc == 0),
                            stop=(dc == DC - 1),
                        )
                    if pending is not None:
                        emit_num(*pending)
                    pq_sb = pqp.tile([P, W], num_dt, tag="pq", name="pq")
                    sa = pqp.tile([P, W], F32, tag="sqa", name="sqa")
                    sb = pqp.tile([P, W], F32, tag="sqb", name="sqb")
                    nc.scalar.activation(sa[:], qp[:], ABS)
                    nc.scalar.activation(sb[:], sa[:], EXP, scale=-1.0)
                    nc.scalar.activation(sa[:], sb[:], LN, bias=1.0)
                    nc.vector.scalar_tensor_tensor(
                        pq_sb[:], qp[:], 0.0, sa[:], MAX, ADD
                    )
                    pending = (ci, o, W, h, pq_sb)
                o += W
            emit_num(*pending)
    return nc


def _get_nc(kc: int, qchunks: tuple) -> bass.Bass:
    key = (PREC, kc, qchunks)
    if key not in _CACHED_NC:
        _CACHED_NC[key] = _build_nc(kc, qchunks)
    return _CACHED_NC[key]


def _qchunk_widths(qtot: int) -> tuple:
    """Split qtot (a multiple of 128) into 512-wide chunks plus at most one
    smaller remainder chunk, remainder LAST so the kernel tail is short."""
    nfull, rem = divmod(qtot, 512)
    w = [512] * nfull
    if rem:
        w.append(rem)
    return tuple(w)


def kernel(query, key, Wq, Wk, Wv, query_padding_mask, key_padding_mask):
    global LAST_EXEC_TIME_NS
    query = np.asarray(query, dtype=np.float32)
    key = np.asarray(key, dtype=np.float32)
    Wq = np.asarray(Wq, dtype=np.float32)
    Wk = np.asarray(Wk, dtype=np.float32)
    Wv = np.asarray(Wv, dtype=np.float32)
    qmask = np.asarray(query_padding_mask)
    kmask = np.asarray(key_padding_mask)

    in_dt = np.float32
    if PREC == "fp16":
        in_dt = np.float16
    elif PREC == "bf16":
        import ml_dtypes

        in_dt = ml_dtypes.bfloat16

    # Compaction: gather unpadded rows, pad to a common (over batches)
    # multiple of 128. Padded rows are zeros with keep=0 so they contribute
    # exactly nothing; query rows are scattered back on host.
    kidxs = [np.flatnonzero(~kmask[n]) for n in range(N)]
    qidxs = [np.flatnonzero(~qmask[n]) for n in range(N)]
    kmax = max(max(len(ix) for ix in kidxs), 1)
    qmax = max(max(len(ix) for ix in qidxs), 1)
    kc = -(-kmax // P)  # key chunks of 128
    ktot = kc * P
    qtot = -(-qmax // P) * P
    qchunks = _qchunk_widths(qtot)

    nc = _get_nc(kc, qchunks)

    in_maps = []
    for c in range(NCORES):
        n, g = c // 2, c % 2
        sl = slice(g * OW, (g + 1) * OW)
        kidx, qidx = kidxs[n], qidxs[n]

        key_c = np.zeros((ktot, D), np.float32)
        key_c[: len(kidx)] = key[n][kidx]
        # kT packing: [p, c*D + dc*128 + j] = key_c[c*128 + j, dc*128 + p]
        kT2 = np.ascontiguousarray(
            key_c.reshape(kc, P, DC, P).transpose(3, 0, 2, 1).reshape(P, kc * D)
        ).astype(in_dt)
        km2 = np.zeros(ktot, np.float32)
        km2[: len(kidx)] = 1.0
        km2 = np.ascontiguousarray(km2.reshape(kc, P).T)

        query_c = np.zeros((qtot, D), np.float32)
        query_c[: len(qidx)] = query[n][qidx]
        # qT packing per chunk: [p, dc*W + j] = query_c[o + j, dc*128 + p]
        blocks = []
        o = 0
        for W in qchunks:
            blocks.append(
                query_c[o : o + W].reshape(W, DC, P).transpose(2, 1, 0).reshape(P, DC * W)
            )
            o += W
        qT2 = np.ascontiguousarray(np.concatenate(blocks, axis=1)).astype(in_dt)
        qm2 = np.zeros(qtot, np.float32)
        qm2[: len(qidx)] = 1.0
        qm2 = np.ascontiguousarray(qm2.reshape(-1, P).T)

        in_maps.append(
            {
                "qT": qT2,
                "kT": kT2,
                "wq": np.ascontiguousarray(Wq[sl, :].T.astype(in_dt)),
                "wk": np.ascontiguousarray(Wk[sl, :].T.astype(in_dt)),
                "wv": np.ascontiguousarray(Wv[sl, :].T.astype(in_dt)),
                "qm": qm2,
                "km": km2,
            }
        )

    res = bu.run_bass_kernel_spmd(
        nc, in_maps, core_ids=list(range(NCORES)), trace=TRACE
    )
    LAST_EXEC_TIME_NS = res.exec_time_ns

    full = np.zeros((N, L, D), dtype=np.float32)
    for c in range(NCORES):
        n, g = c // 2, c % 2
        qidx = qidxs[n]
        full[n, qidx, g * OW : (g + 1) * OW] = res.results[c]["out"][: len(qidx)]
    return full


# revision 11
# speedup vs baseline: 1.1042x; 1.0043x over previous
"""Trainium2 Bass kernel for nn_MultiHeadAttention_89429809037632.

Linear attention (softplus feature map) with padding masks:
    q = query @ Wq.T ; k = key @ Wk.T ; v = key @ Wv.T   (per-head split)
    pq = softplus(q) ; pk = softplus(k) * keep(key_mask)
    kv = pk^T v (per head, plus a fused ones-column giving sum(pk))
    out = (pq @ kv) / (pq @ sum(pk)) * keep(query_mask)

Sharding across 8 NeuronCores: data-parallel over N=4 batches x
tensor-parallel over 2 head-groups (8 heads x 128 dims = 1024 output
dims each). Host transposes activations/weights so the contraction
dim (D) lands on the SBUF partition axis; each core runs an identical
SPMD program on its shard, outputs are concatenated on host.

Padding compaction: ~10% of keys/queries are padded (masked to zero
contribution / zero output). The host gathers the unpadded rows,
pads to a common 128-multiple across batches (compiled shapes depend
only on the rounded counts, cached), and scatters the output back.
Removed keys contribute exactly 0 (pk=0) so numerics are unchanged;
this cuts ~9% of all three projection GEMMs (PE issue time is the
bottleneck at 94% tensor-engine occupancy).

Per-core program (Tile framework), fp16 inputs with fp32 PSUM
accumulation (measured ~4.4e-4 scale-relative absmax):
  Warmup: ~96 dependency-free dummy matmuls issued at t=0 keep the PE
    busy while the first DMAs land, flipping the HAM clock gate from
    1.2GHz to 2.4GHz before real work starts.
  Phase A: for each 128-key chunk: project K,V (full-rate matmuls,
    stationary = key^T d-chunk serving 4 matmuls), softplus+mask -> pk,
    copy V into a [v | 1] block layout, then 8 per-head matmuls
    accumulate kv_aug (128x129 per head) in PSUM across all chunks.
  Phase B: for each query chunk (512-query chunks + one remainder) x
    head: project Q, softplus -> pq, then per 128-query subchunk one
    matmul against kv_aug gives [num | den]; epilogue computes
    num * (keep/den) on DVE into a per-chunk staging tile shipped as
    one chunk-sized DMA.
  kT/qT are host-packed so each chunk tile loads with ONE dma_start of
  4-16KB contiguous per-partition runs. DMA descriptor generation is
  spread over three rings (kt+out on sync, wk+wq+qt on gpsimd, wv on
  scalar) so the opening weight stream isn't serialized behind one
  sequencer. Matmul emission is software-pipelined (kv/num matmuls
  trail their producer chunk by one step).
"""

import json
import os
import sys
import types

import numpy as np

for _p in ("/opt/trn_rl_repo",):
    if _p not in sys.path and os.path.isdir(_p):
        sys.path.insert(0, _p)

# ``run_bass_kernel_spmd(trace=True)`` imports antenv.axon_hooks, which not
# every image ships. Provide a stub so the import never crashes (returning
# None simply disables NTFF tracing).
try:
    import antenv.axon_hooks  # noqa: F401
except Exception:
    try:
        import antenv

        _m = types.ModuleType("antenv.axon_hooks")
        _HOOK = [None]

        def _get_hook():
            if _HOOK[0] is None:
                try:
                    from trn_agent_boot.trn_boot import _ntff_profile_via_ctypes

                    _HOOK[0] = _ntff_profile_via_ctypes("/opt/axon/libaxon_pjrt.so")
                except Exception:
                    _HOOK[0] = None
            return _HOOK[0]

        _m.get_axon_ntff_profile_hook = _get_hook
        _m.set_axon_ntff_profile_hook = lambda h: _HOOK.__setitem__(0, h)
        sys.modules["antenv.axon_hooks"] = _m
        antenv.axon_hooks = _m
    except Exception:
        pass

import concourse.bass as bass
import concourse.bass_utils as bu
import concourse.mybir as mybir
import concourse.tile as tile

# ---------------------------------------------------------------------------
# Shim 1: this container's walrus accepts only ONE sync-wait per instruction
# ("Too many sync wait commands"); Tile attaches several. Rewrite the BIR
# JSON so excess waits ride on same-engine NoOps immediately before the
# instruction (engine streams are in-order, so this is equivalent).
# Shim 2: upload_artifacts wants a cloud bucket; keep artifacts local.
# ---------------------------------------------------------------------------
_MAX_WAITS = 1


def _split_multi_waits(bir_bytes: bytes) -> bytes:
    d = json.loads(bir_bytes)
    ctr = 0
    changed = False
    for fn in d.get("functions", []):
        for bb in fn.get("blocks", []):
            out = []
            for inst in bb.get("instructions", []):
                si = inst.get("sync_info")
                waits = (si or {}).get("on_wait") or []
                if len(waits) > _MAX_WAITS:
                    changed = True
                    idx = 0
                    while len(waits) - idx > _MAX_WAITS:
                        chunk = waits[idx : idx + _MAX_WAITS]
                        idx += _MAX_WAITS
                        ctr += 1
                        nop = {
                            "engine": inst["engine"],
                            "ins": [],
                            "outs": [],
                            "name": f"I-wsplit-{ctr}",
                            "opcode": "NoOp",
                            "sync_info": {"on_update": [], "on_wait": chunk},
                        }
                        if "debug" in inst:
                            nop["debug"] = inst["debug"]
                        out.append(nop)
                    si["on_wait"] = waits[idx:]
                out.append(inst)
            bb["instructions"] = out
    return json.dumps(d).encode() if changed else bir_bytes


if not getattr(bass.Bass, "_wait_split_shim", False):
    _orig_to_json = bass.Bass.to_json_bytes

    def _to_json_bytes(self) -> bytes:
        return _split_multi_waits(_orig_to_json(self))

    bass.Bass.to_json_bytes = _to_json_bytes
    bass.Bass._wait_split_shim = True
    bu.upload_artifacts = lambda tmpdir: tmpdir

# ---------------------------------------------------------------------------
# Problem shapes (hardcoded per contract)
# ---------------------------------------------------------------------------
N, L, D = 4, 4096, 2048  # batches, seq len (q and k), model dim
H, P = 16, 128  # heads, head dim
NCORES = 8
HL = H // 2  # heads per core (head-group of 8)
OW = HL * P  # per-core projected width (1024)
DC = D // P  # 16 contraction chunks

F32 = mybir.dt.float32
F32R = mybir.dt.float32r
# The ACT tables in this walrus build ship no softplus; synthesize the
# numerically stable form softplus(x) = max(x,0) + ln(1 + exp(-|x|)) from
# set 6 ("natural_log_exp_and_others": abs/exp/ln in one resident table).
ABS = mybir.ActivationFunctionType.Abs
EXP = mybir.ActivationFunctionType.Exp
LN = mybir.ActivationFunctionType.Ln
MUL = mybir.AluOpType.mult
MAX = mybir.AluOpType.max
ADD = mybir.AluOpType.add

# kv_aug per-head column offsets inside the 3-bank PSUM accumulator:
# 3 heads per 2 KiB bank (129 fp32 columns each, none crossing a bank edge).
_KV_BASE = [(h // 3) * 512 + (h % 3) * 129 for h in range(HL)]

TRACE = False  # set True (e.g. from test.py) to capture NTFF profile
LAST_EXEC_TIME_NS = None

# Precision mode for all matmuls:
#   "f32r": projections in float32r (full rate, ~1.6e-4 end rel err),
#           attention stage fp32 (1/4-rate small matmuls, widened f32r num).
#   "fp16": everything fp16 inputs + fp32 PSUM accumulation (full rate,
#           ~11-bit input rounding => ~5e-4 end rel err, fastest).
#   "bf16": like fp16 but 8-bit mantissa (~2e-3 end rel err).
PREC = "fp16"

N_WARM = 96  # dummy matmuls to warm the HAM clock gate while DMAs land

_CACHED_NC = {}


def _build_nc(kc: int, qchunks: tuple) -> bass.Bass:
    """Build the per-core program for kc 128-key chunks and the given
    query chunk widths (each a multiple of 128; 512 in steady state)."""
    from contextlib import ExitStack

    qtot = sum(qchunks)
    qcb = qtot // P  # query keep-mask columns

    lp = {"fp16": mybir.dt.float16, "bf16": mybir.dt.bfloat16}.get(PREC)
    IN_DT = F32R if lp is None else lp  # projections (dram + sbuf operands)
    att_dt = F32 if lp is None else lp  # kv-stage operands (pk, v_aug)
    # num-stage matmul: fp16/bf16 run full-rate at any width; the f32r path
    # widens to 258 moving columns (>=256 streams at 1 cyc/row vs fp32's 4)
    # — the upper 129 columns are discarded junk.
    num_dt = F32R if lp is None else lp
    NW = 258 if lp is None else 129
    KV_W = HL * 129 + (129 if lp is None else 0)  # pad so h=7 reads 258 cols
    # fp16 output staging+DMA (host converts back): halves the output
    # traffic and the kernel-tail DMA; adds ~2.4e-4 relative rounding.
    OUT_DT = F32 if lp is None else mybir.dt.float16

    nc = bass.Bass()
    # qT/kT are host-packed: column block c holds that chunk's activations
    # with layout [p, dc*W + j] = act[chunk_off + j, dc*128 + p], so a chunk
    # loads as ONE dma_start with a contiguous per-partition run and the
    # dc-slices come out as plain column slices.
    qT = nc.dram_tensor("qT", (P, DC * qtot), IN_DT, kind="ExternalInput")
    kT = nc.dram_tensor("kT", (P, kc * D), IN_DT, kind="ExternalInput")
    wq = nc.dram_tensor("wq", (D, OW), IN_DT, kind="ExternalInput")
    wk = nc.dram_tensor("wk", (D, OW), IN_DT, kind="ExternalInput")
    wv = nc.dram_tensor("wv", (D, OW), IN_DT, kind="ExternalInput")
    qm = nc.dram_tensor("qm", (P, qcb), F32, kind="ExternalInput")
    km = nc.dram_tensor("km", (P, kc), F32, kind="ExternalInput")
    out = nc.dram_tensor("out", (qtot, OW), OUT_DT, kind="ExternalOutput")

    with tile.TileContext(nc) as tc, ExitStack() as outer:
        # ------ PE warmup: the HAM clock gate starts at 1.2GHz and needs
        # ~3.4us of sustained activity to release to 2.4GHz. The opening
        # DMAs take ~10us to land; fill that window with dependency-free
        # matmuls so real work starts warm. Pool closes to free the bank.
        with (
            tc.tile_pool(name="warm", bufs=1) as warmp,
            tc.tile_pool(name="warmps", bufs=1, space="PSUM") as warmpsp,
        ):
            wt = warmp.tile([P, P], IN_DT, name="wt")
            nc.vector.memset(wt[:], 0.0)
            wps = warmpsp.tile([P, P], F32, name="wps")
            for _ in range(N_WARM):
                nc.tensor.matmul(wps[:], wt[:], wt[:], start=True, stop=True)

        misc = outer.enter_context(tc.tile_pool(name="misc", bufs=1))
        kvpool = outer.enter_context(tc.tile_pool(name="kvsb", bufs=1))
        # qt pool lives at OUTER scope so its SBUF region is disjoint from
        # the phase-A pools: the first chunks' qt DMAs (ring-ahead on the
        # gpsimd queue) then stream in during phase A with no anti-deps.
        qtp = outer.enter_context(tc.tile_pool(name="qt", bufs=2))
        wqp = outer.enter_context(tc.tile_pool(name="wqe", bufs=1))
        qm_sb = misc.tile([P, qcb], F32)
        km_sb = misc.tile([P, kc], F32)
        nc.scalar.dma_start(qm_sb[:], qm[:])
        nc.scalar.dma_start(km_sb[:], km[:])
        kv_sb = kvpool.tile([P, KV_W], num_dt)

        # ------ Phase A: K/V projection + kv accumulation ------------------
        # One pass, all 8 heads. Each kt d-chunk slice serves 4 consecutive
        # matmuls (K and V, two 512-wide o-halves each).
        esA = ExitStack()
        wkvp = esA.enter_context(tc.tile_pool(name="wkv", bufs=1))
        ktp = esA.enter_context(tc.tile_pool(name="kt", bufs=3))
        pkp = esA.enter_context(tc.tile_pool(name="pk", bufs=3))
        vap = esA.enter_context(tc.tile_pool(name="vaug", bufs=3))
        pps = esA.enter_context(tc.tile_pool(name="projps", bufs=5, space="PSUM"))
        kvps = esA.enter_context(tc.tile_pool(name="kvps", bufs=1, space="PSUM"))
        kv_ps = kvps.tile([P, 1536], F32)

        # kt chunk 0 FIRST on the sync ring so the opening matmuls wait on
        # 0.5 MB; weights stream concurrently on the gpsimd + scalar rings.
        kt0 = ktp.tile([P, D], IN_DT, tag="kt", name="kt0")
        nc.sync.dma_start(kt0[:], kT[:, 0:D])

        wk_sb = [
            wkvp.tile([P, OW], IN_DT, tag=f"wk{dc}", name=f"wk{dc}")
            for dc in range(DC)
        ]
        wv_sb = [
            wkvp.tile([P, OW], IN_DT, tag=f"wv{dc}", name=f"wv{dc}")
            for dc in range(DC)
        ]
        for dc in range(DC):
            nc.gpsimd.dma_start(wk_sb[dc][:], wk[dc * P : (dc + 1) * P, :])
            nc.scalar.dma_start(wv_sb[dc][:], wv[dc * P : (dc + 1) * P, :])
        # wq tiles are declared now but their DMAs are deferred until the
        # opening chunks have consumed wk/wv: the DMA engines are a shared
        # ~358 GB/s pool, and letting the (not-yet-needed) wq/qt stream run
        # early starves the critical phase-A weight loads.
        wq_sb = [
            wqp.tile([P, OW], IN_DT, tag=f"wq{dc}", name=f"wq{dc}")
            for dc in range(DC)
        ]

        bank_start = {}

        def emit_kv_mms(c, pk_sb, va_sb):
            for h in range(HL):
                bank_first = h % 3 == 0
                mm = nc.tensor.matmul(
                    kv_ps[:, _KV_BASE[h] : _KV_BASE[h] + 129],
                    pk_sb[:, h * P : (h + 1) * P],
                    va_sb[:, h * 129 : (h + 1) * 129],
                    start=(c == 0 and bank_first),
                    stop=(c == kc - 1),
                    skip_group_check=True,
                )
                if c == 0:
                    # start=True clears has_written for the whole PSUM bank;
                    # siblings must come after their bank's clear.
                    if bank_first:
                        bank_start[h // 3] = mm
                    else:
                        tile.add_dep_helper(
                            mm.ins,
                            bank_start[h // 3].ins,
                            reason="kv bank has_written clear order",
                        )

        # kv matmuls for chunk c are emitted after chunk c+1's projection
        # matmuls: their pk operand is only ready ~3us after chunk c's last
        # projection, so this keeps PE fed meanwhile.
        pending = None
        pps_allocs = 0
        wq_anchor = None
        wq_emitted = False
        for c in range(kc):
            if c == 0:
                kt_sb = kt0
            else:
                kt_sb = ktp.tile([P, D], IN_DT, tag="kt", name=f"kt{c}")
                nc.sync.dma_start(kt_sb[:], kT[:, c * D : (c + 1) * D])
            if not wq_emitted and wq_anchor is not None:
                # release the deferred wq stream (qt chunks ride behind it
                # on the same in-order gpsimd ring)
                dma = nc.gpsimd.dma_start(wq_sb[0][:], wq[0:P, :])
                tile.add_dep_helper(
                    dma.ins, wq_anchor.ins, reason="defer wq behind wk/wv"
                )
                for dc in range(1, DC):
                    nc.gpsimd.dma_start(
                        wq_sb[dc][:], wq[dc * P : (dc + 1) * P, :]
                    )
                wq_emitted = True
            if c == kc - 1:
                # rotate the proj-PSUM allocation so the final chunk's tiles
                # land on slots 1..4 and slot 0 (which phase B's first qp
                # tile will alias) was last touched two chunks earlier —
                # phase B's opening matmul then has no WAR stall.
                for _ in range((1 - pps_allocs) % 5):
                    pps.tile([P, 512], F32, tag="proj", name="spacer")
                    pps_allocs += 1
            kp0 = pps.tile([P, 512], F32, tag="proj", name="kp0")
            kp1 = pps.tile([P, 512], F32, tag="proj", name="kp1")
            vp0 = pps.tile([P, 512], F32, tag="proj", name="vp0")
            vp1 = pps.tile([P, 512], F32, tag="proj", name="vp1")
            pps_allocs += 4
            for dc in range(DC):
                lhsT = kt_sb[:, dc * P : (dc + 1) * P]
                st = dict(start=(dc == 0), stop=(dc == DC - 1))
                nc.tensor.matmul(kp0[:], lhsT, wk_sb[dc][:, 0:512], **st)
                nc.tensor.matmul(kp1[:], lhsT, wk_sb[dc][:, 512:1024], **st)
                nc.tensor.matmul(vp0[:], lhsT, wv_sb[dc][:, 0:512], **st)
                mmv = nc.tensor.matmul(vp1[:], lhsT, wv_sb[dc][:, 512:1024], **st)
                if c == min(2, kc - 1) and dc == DC - 1:
                    wq_anchor = mmv

            if pending is not None:
                emit_kv_mms(*pending)

            pk_sb = pkp.tile([P, OW], att_dt, tag="pk", name="pk")
            for half, kp in ((0, kp0), (1, kp1)):
                sa = pkp.tile([P, 512], F32, tag="sa", name="sa")
                sb = pkp.tile([P, 512], F32, tag="sb", name="sb")
                nc.scalar.activation(sa[:], kp[:], ABS)
                nc.scalar.activation(sb[:], sa[:], EXP, scale=-1.0)
                nc.scalar.activation(sa[:], sb[:], LN, bias=1.0)
                nc.vector.scalar_tensor_tensor(
                    pk_sb[:, half * 512 : (half + 1) * 512],
                    kp[:],
                    0.0,
                    sa[:],
                    MAX,
                    ADD,
                )
            nc.vector.tensor_scalar_mul(pk_sb[:], pk_sb[:], km_sb[:, c : c + 1])

            va_sb = vap.tile([P, HL * 129], att_dt, tag="vaug", name="va")
            nc.vector.memset(
                va_sb[:].rearrange("p (h x) -> p h x", x=129)[:, :, 128:129], 1.0
            )
            for h in range(HL):
                src = vp0 if h < 4 else vp1
                off = (h % 4) * P
                nc.vector.tensor_copy(
                    va_sb[:, h * 129 : h * 129 + P], src[:, off : off + P]
                )
            pending = (c, pk_sb, va_sb)

        if not wq_emitted:
            dma = nc.gpsimd.dma_start(wq_sb[0][:], wq[0:P, :])
            tile.add_dep_helper(
                dma.ins, wq_anchor.ins, reason="defer wq behind wk/wv"
            )
            for dc in range(1, DC):
                nc.gpsimd.dma_start(wq_sb[dc][:], wq[dc * P : (dc + 1) * P, :])

        # Dependency-free bridge matmuls: the final chunk's kv matmuls wait
        # ~2.5us for its softplus, and the in-order PE queue would idle.
        # These land on proj-PSUM slot 0 (free since two chunks ago, thanks
        # to the spacer rotation) and read the resident kt tile, so they
        # issue immediately and keep the PE busy+warm across the boundary.
        bridge = pps.tile([P, 512], F32, tag="proj", name="bridge")
        pps_allocs += 1
        for _ in range(14):
            nc.tensor.matmul(
                bridge[:], kt_sb[:, 0:P], kt_sb[:, 0:512], start=True, stop=True
            )
        emit_kv_mms(*pending)

        for h in range(HL):
            nc.vector.tensor_copy(
                kv_sb[:, h * 129 : (h + 1) * 129],
                kv_ps[:, _KV_BASE[h] : _KV_BASE[h] + 129],
            )
        if KV_W > HL * 129:
            # f32r-typed zero pad (junk columns read by head 7's widened MM);
            # written by a DVE op so the FP32R-rounding verifier is satisfied.
            nc.vector.tensor_scalar_mul(
                kv_sb[:, HL * 129 : KV_W], kv_sb[:, 0:129], 0.0
            )
        esA.close()

        # ------ Phase B: Q projection + attention epilogue -----------------
        with (
            tc.tile_pool(name="pq", bufs=4) as pqp,
            tc.tile_pool(name="sc", bufs=4) as scp,
            tc.tile_pool(name="st", bufs=3) as stp,
            tc.tile_pool(name="qpps", bufs=4, space="PSUM") as qpps,
            tc.tile_pool(name="nmps", bufs=4, space="PSUM") as nmps,
        ):
            st_tiles = {}

            def emit_num(ci, o, W, h, pq_sb):
                # results stage into st (partition=l%128, cols j*OW+o) and
                # ship as ONE multi-run DMA per chunk — per-(h,j) output
                # DMAs cost ~0.6us of descriptor generation each.
                nj = W // P
                if h == 0:
                    st_tiles[ci] = stp.tile(
                        [P, nj * OW], OUT_DT, tag="st", name="st"
                    )
                st = st_tiles[ci]
                for j in range(nj):
                    nm = nmps.tile([P, NW], F32, tag="nm", name="nm")
                    nc.tensor.matmul(
                        nm[:],
                        pq_sb[:, j * P : (j + 1) * P],
                        kv_sb[:, h * 129 : h * 129 + NW],
                        start=True,
                        stop=True,
                    )
                    sc = scp.tile([P, 1], F32, tag="sc", name="sc")
                    nc.vector.reciprocal(sc[:], nm[:, 128:129])
                    col = o // P + j
                    nc.vector.tensor_scalar(
                        st[:, j * OW + h * P : j * OW + (h + 1) * P],
                        nm[:, 0:P],
                        sc[:, 0:1],
                        qm_sb[:, col : col + 1],
                        MUL,
                        MUL,
                    )
                if h == HL - 1:
                    nc.sync.dma_start(
                        out[o : o + W, :].rearrange("(j p) o -> p j o", p=P),
                        st[:].rearrange("p (j o) -> p j o", o=OW),
                    )
                    del st_tiles[ci]

            # num matmuls for step (ci,h) are emitted two steps behind the
            # projection matmuls (pq is ~2.5us of ACT/DVE behind qp; depth-2
            # keeps the PE fed through the boundary where the ACT queue is
            # still draining phase A's last softplus).
            from collections import deque

            pendq = deque()
            o = 0
            for ci, W in enumerate(qchunks):
                qt_sb = qtp.tile([P, DC * W], IN_DT, tag="qt", name=f"qt{ci}")
                nc.gpsimd.dma_start(
                    qt_sb[:], qT[:, DC * o : DC * (o + W)]
                )
                for h in range(HL):
                    qp = qpps.tile([P, W], F32, tag="qp", name="qp")
                    for dc in range(DC):
                        nc.tensor.matmul(
                            qp[:],
                            wq_sb[dc][:, h * P : (h + 1) * P],
                            qt_sb[:, dc * W : (dc + 1) * W],
                            start=(dc == 0),
                            stop=(dc == DC - 1),
                        )
                    if len(pendq) >= 2:
                        emit_num(*pendq.popleft())
                    pq_sb = pqp.tile([P, W], num_dt, tag="pq", name="pq")
                    sa = pqp.tile([P, W], F32, tag="sqa", name="sqa")
                    sb = pqp.tile([P, W], F32, tag="sqb", name="sqb")
                    nc.scalar.activation(sa[:], qp[:], ABS)
                    nc.scalar.activation(sb[:], sa[:], EXP, scale=-1.0)
                    nc.scalar.activation(sa[:], sb[:], LN, bias=1.0)
                    nc.vector.scalar_tensor_tensor(
                        pq_sb[:], qp[:], 0.0, sa[:], MAX, ADD
                    )
                    pendq.append((ci, o, W, h, pq_sb))
                o += W
            while pendq:
                emit_num(*pendq.popleft())
    return nc


def _get_nc(kc: int, qchunks: tuple) -> bass.Bass:
    key = (PREC, kc, qchunks)
    if key not in _CACHED_NC:
        _CACHED_NC[key] = _build_nc(kc, qchunks)
    return _CACHED_NC[key]


def _qchunk_widths(qtot: int) -> tuple:
    """Split qtot (a multiple of 128) into 512-wide chunks plus at most one
    smaller remainder chunk, remainder LAST so the kernel tail is short."""
    nfull, rem = divmod(qtot, 512)
    w = [512] * nfull
    if rem:
        w.append(rem)
    return tuple(w)


def kernel(query, key, Wq, Wk, Wv, query_padding_mask, key_padding_mask):
    global LAST_EXEC_TIME_NS
    query = np.asarray(query, dtype=np.float32)
    key = np.asarray(key, dtype=np.float32)
    Wq = np.asarray(Wq, dtype=np.float32)
    Wk = np.asarray(Wk, dtype=np.float32)
    Wv = np.asarray(Wv, dtype=np.float32)
    qmask = np.asarray(query_padding_mask)
    kmask = np.asarray(key_padding_mask)

    in_dt = np.float32
    if PREC == "fp16":
        in_dt = np.float16
    elif PREC == "bf16":
        import ml_dtypes

        in_dt = ml_dtypes.bfloat16

    # Compaction: gather unpadded rows, pad to a common (over batches)
    # multiple of 128. Padded rows are zeros with keep=0 so they contribute
    # exactly nothing; query rows are scattered back on host.
    kidxs = [np.flatnonzero(~kmask[n]) for n in range(N)]
    qidxs = [np.flatnonzero(~qmask[n]) for n in range(N)]
    kmax = max(max(len(ix) for ix in kidxs), 1)
    qmax = max(max(len(ix) for ix in qidxs), 1)
    kc = -(-kmax // P)  # key chunks of 128
    ktot = kc * P
    qtot = -(-qmax // P) * P
    qchunks = _qchunk_widths(qtot)

    nc = _get_nc(kc, qchunks)

    in_maps = []
    for c in range(NCORES):
        n, g = c // 2, c % 2
        sl = slice(g * OW, (g + 1) * OW)
        kidx, qidx = kidxs[n], qidxs[n]

        key_c = np.zeros((ktot, D), np.float32)
        key_c[: len(kidx)] = key[n][kidx]
        # kT packing: [p, c*D + dc*128 + j] = key_c[c*128 + j, dc*128 + p]
        kT2 = np.ascontiguousarray(
            key_c.reshape(kc, P, DC, P).transpose(3, 0, 2, 1).reshape(P, kc * D)
        ).astype(in_dt)
        km2 = np.zeros(ktot, np.float32)
        km2[: len(kidx)] = 1.0
        km2 = np.ascontiguousarray(km2.reshape(kc, P).T)

        query_c = np.zeros((qtot, D), np.float32)
        query_c[: len(qidx)] = query[n][qidx]
        # qT packing per chunk: [p, dc*W + j] = query_c[o + j, dc*128 + p]
        blocks = []
        o = 0
        for W in qchunks:
            blocks.append(
                query_c[o : o + W].reshape(W, DC, P).transpose(2, 1, 0).reshape(P, DC * W)
            )
            o += W
        qT2 = np.ascontiguousarray(np.concatenate(blocks, axis=1)).astype(in_dt)
        qm2 = np.zeros(qtot, np.float32)
        qm2[: len(qidx)] = 1.0
        qm2 = np.ascontiguousarray(qm2.reshape(-1, P).T)

        in_maps.append(
            {
                "qT": qT2,
                "kT": kT2,
                "wq": np.ascontiguousarray(Wq[sl, :].T.astype(in_dt)),
                "wk": np.ascontiguousarray(Wk[sl, :].T.astype(in_dt)),
                "wv": np.ascontiguousarray(Wv[sl, :].T.astype(in_dt)),
                "qm": qm2,
                "km": km2,
            }
        )

    res = bu.run_bass_kernel_spmd(
        nc, in_maps, core_ids=list(range(NCORES)), trace=TRACE
    )
    LAST_EXEC_TIME_NS = res.exec_time_ns

    full = np.zeros((N, L, D), dtype=np.float32)
    for c in range(NCORES):
        n, g = c // 2, c % 2
        qidx = qidxs[n]
        full[n, qidx, g * OW : (g + 1) * OW] = res.results[c]["out"][: len(qidx)]
    return full


# revision 16
# speedup vs baseline: 1.1077x; 1.0031x over previous
"""Trainium2 Bass kernel for nn_MultiHeadAttention_89429809037632.

Linear attention (softplus feature map) with padding masks:
    q = query @ Wq.T ; k = key @ Wk.T ; v = key @ Wv.T   (per-head split)
    pq = softplus(q) ; pk = softplus(k) * keep(key_mask)
    kv = pk^T v (per head, plus a fused ones-column giving sum(pk))
    out = (pq @ kv) / (pq @ sum(pk)) * keep(query_mask)

Sharding across 8 NeuronCores: data-parallel over N=4 batches x
tensor-parallel over 2 head-groups (8 heads x 128 dims = 1024 output
dims each). Host transposes activations/weights so the contraction
dim (D) lands on the SBUF partition axis; each core runs an identical
SPMD program on its shard, outputs are concatenated on host.

Padding compaction: ~10% of keys/queries are padded (masked to zero
contribution / zero output). The host gathers the unpadded rows,
pads to a common 128-multiple across batches (compiled shapes depend
only on the rounded counts, cached), and scatters the output back.
Removed keys contribute exactly 0 (pk=0) so numerics are unchanged;
this cuts ~9% of all three projection GEMMs (PE issue time is the
bottleneck at 94% tensor-engine occupancy).

Per-core program (Tile framework), fp16 inputs with fp32 PSUM
accumulation (measured ~4.4e-4 scale-relative absmax):
  Warmup: ~96 dependency-free dummy matmuls issued at t=0 keep the PE
    busy while the first DMAs land, flipping the HAM clock gate from
    1.2GHz to 2.4GHz before real work starts.
  Phase A: for each 128-key chunk: project K,V (full-rate matmuls,
    stationary = key^T d-chunk serving 4 matmuls), softplus+mask -> pk,
    copy V into a [v | 1] block layout, then 8 per-head matmuls
    accumulate kv_aug (128x129 per head) in PSUM across all chunks.
  Phase B: for each query chunk (512-query chunks + one remainder) x
    head: project Q, softplus -> pq, then per 128-query subchunk one
    matmul against kv_aug gives [num | den]; epilogue computes
    num * (keep/den) on DVE into a per-chunk staging tile shipped as
    one chunk-sized DMA.
  kT/qT are host-packed so each chunk tile loads with ONE dma_start of
  4-16KB contiguous per-partition runs. DMA descriptor generation is
  spread over three rings (kt+out on sync, wk+wq+qt on gpsimd, wv on
  scalar) so the opening weight stream isn't serialized behind one
  sequencer. Matmul emission is software-pipelined (kv/num matmuls
  trail their producer chunk by one step).
"""

import json
import os
import sys
import types

import numpy as np

for _p in ("/opt/trn_rl_repo",):
    if _p not in sys.path and os.path.isdir(_p):
        sys.path.insert(0, _p)

# ``run_bass_kernel_spmd(trace=True)`` imports antenv.axon_hooks, which not
# every image ships. Provide a stub so the import never crashes (returning
# None simply disables NTFF tracing).
try:
    import antenv.axon_hooks  # noqa: F401
except Exception:
    try:
        import antenv

        _m = types.ModuleType("antenv.axon_hooks")
        _HOOK = [None]

        def _get_hook():
            if _HOOK[0] is None:
                try:
                    from trn_agent_boot.trn_boot import _ntff_profile_via_ctypes

                    _HOOK[0] = _ntff_profile_via_ctypes("/opt/axon/libaxon_pjrt.so")
                except Exception:
                    _HOOK[0] = None
            return _HOOK[0]

        _m.get_axon_ntff_profile_hook = _get_hook
        _m.set_axon_ntff_profile_hook = lambda h: _HOOK.__setitem__(0, h)
        sys.modules["antenv.axon_hooks"] = _m
        antenv.axon_hooks = _m
    except Exception:
        pass

import concourse.bass as bass
import concourse.bass_utils as bu
import concourse.mybir as mybir
import concourse.tile as tile

# ---------------------------------------------------------------------------
# Shim 1: this container's walrus accepts only ONE sync-wait per instruction
# ("Too many sync wait commands"); Tile attaches several. Rewrite the BIR
# JSON so excess waits ride on same-engine NoOps immediately before the
# instruction (engine streams are in-order, so this is equivalent).
# Shim 2: upload_artifacts wants a cloud bucket; keep artifacts local.
# ---------------------------------------------------------------------------
_MAX_WAITS = 1


def _split_multi_waits(bir_bytes: bytes) -> bytes:
    d = json.loads(bir_bytes)
    ctr = 0
    changed = False
    for fn in d.get("functions", []):
        for bb in fn.get("blocks", []):
            out = []
            for inst in bb.get("instructions", []):
                si = inst.get("sync_info")
                waits = (si or {}).get("on_wait") or []
                if len(waits) > _MAX_WAITS:
                    changed = True
                    idx = 0
                    while len(waits) - idx > _MAX_WAITS:
                        chunk = waits[idx : idx + _MAX_WAITS]
                        idx += _MAX_WAITS
                        ctr += 1
                        nop = {
                            "engine": inst["engine"],
                            "ins": [],
                            "outs": [],
                            "name": f"I-wsplit-{ctr}",
                            "opcode": "NoOp",
                            "sync_info": {"on_update": [], "on_wait": chunk},
                        }
                        if "debug" in inst:
                            nop["debug"] = inst["debug"]
                        out.append(nop)
                    si["on_wait"] = waits[idx:]
                out.append(inst)
            bb["instructions"] = out
    return json.dumps(d).encode() if changed else bir_bytes


if not getattr(bass.Bass, "_wait_split_shim", False):
    _orig_to_json = bass.Bass.to_json_bytes

    def _to_json_bytes(self) -> bytes:
        return _split_multi_waits(_orig_to_json(self))

    bass.Bass.to_json_bytes = _to_json_bytes
    bass.Bass._wait_split_shim = True
    bu.upload_artifacts = lambda tmpdir: tmpdir

# ---------------------------------------------------------------------------
# Problem shapes (hardcoded per contract)
# ---------------------------------------------------------------------------
N, L, D = 4, 4096, 2048  # batches, seq len (q and k), model dim
H, P = 16, 128  # heads, head dim
NCORES = 8
HL = H // 2  # heads per core (head-group of 8)
OW = HL * P  # per-core projected width (1024)
DC = D // P  # 16 contraction chunks

F32 = mybir.dt.float32
F32R = mybir.dt.float32r
# The ACT tables in this walrus build ship no softplus; synthesize the
# numerically stable form softplus(x) = max(x,0) + ln(1 + exp(-|x|)) from
# set 6 ("natural_log_exp_and_others": abs/exp/ln in one resident table).
ABS = mybir.ActivationFunctionType.Abs
EXP = mybir.ActivationFunctionType.Exp
LN = mybir.ActivationFunctionType.Ln
MUL = mybir.AluOpType.mult
MAX = mybir.AluOpType.max
ADD = mybir.AluOpType.add

# kv_aug per-head column offsets inside the 3-bank PSUM accumulator:
# 3 heads per 2 KiB bank (129 fp32 columns each, none crossing a bank edge).
_KV_BASE = [(h // 3) * 512 + (h % 3) * 129 for h in range(HL)]

TRACE = False  # set True (e.g. from test.py) to capture NTFF profile
LAST_EXEC_TIME_NS = None

# Precision mode for all matmuls:
#   "f32r": projections in float32r (full rate, ~1.6e-4 end rel err),
#           attention stage fp32 (1/4-rate small matmuls, widened f32r num).
#   "fp16": everything fp16 inputs + fp32 PSUM accumulation (full rate,
#           ~11-bit input rounding => ~5e-4 end rel err, fastest).
#   "bf16": like fp16 but 8-bit mantissa (~2e-3 end rel err).
PREC = "fp16"

N_WARM = 96  # dummy matmuls to warm the HAM clock gate while DMAs land

_CACHED_NC = {}


def _build_nc(kc: int, qchunks: tuple) -> bass.Bass:
    """Build the per-core program for kc 128-key chunks and the given
    query chunk widths (each a multiple of 128; 512 in steady state)."""
    from contextlib import ExitStack

    qtot = sum(qchunks)
    qcb = qtot // P  # query keep-mask columns

    lp = {"fp16": mybir.dt.float16, "bf16": mybir.dt.bfloat16}.get(PREC)
    IN_DT = F32R if lp is None else lp  # projections (dram + sbuf operands)
    att_dt = F32 if lp is None else lp  # kv-stage operands (pk, v_aug)
    # num-stage matmul: fp16/bf16 run full-rate at any width; the f32r path
    # widens to 258 moving columns (>=256 streams at 1 cyc/row vs fp32's 4)
    # — the upper 129 columns are discarded junk.
    num_dt = F32R if lp is None else lp
    NW = 258 if lp is None else 129
    KV_W = HL * 129 + (129 if lp is None else 0)  # pad so h=7 reads 258 cols
    # fp16 output staging+DMA (host converts back): halves the output
    # traffic and the kernel-tail DMA; adds ~2.4e-4 relative rounding.
    OUT_DT = F32 if lp is None else mybir.dt.float16

    nc = bass.Bass()
    # qT/kT are host-packed: column block c holds that chunk's activations
    # with layout [p, dc*W + j] = act[chunk_off + j, dc*128 + p], so a chunk
    # loads as ONE dma_start with a contiguous per-partition run and the
    # dc-slices come out as plain column slices.
    qT = nc.dram_tensor("qT", (P, DC * qtot), IN_DT, kind="ExternalInput")
    kT = nc.dram_tensor("kT", (P, kc * D), IN_DT, kind="ExternalInput")
    wq = nc.dram_tensor("wq", (D, OW), IN_DT, kind="ExternalInput")
    wk = nc.dram_tensor("wk", (D, OW), IN_DT, kind="ExternalInput")
    wv = nc.dram_tensor("wv", (D, OW), IN_DT, kind="ExternalInput")
    qm = nc.dram_tensor("qm", (P, qcb), F32, kind="ExternalInput")
    km = nc.dram_tensor("km", (P, kc), F32, kind="ExternalInput")
    out = nc.dram_tensor("out", (qtot, OW), OUT_DT, kind="ExternalOutput")

    with tile.TileContext(nc) as tc, ExitStack() as outer:
        # ------ PE warmup: the HAM clock gate starts at 1.2GHz and needs
        # ~3.4us of sustained activity to release to 2.4GHz. The opening
        # DMAs take ~10us to land; fill that window with dependency-free
        # matmuls so real work starts warm. Pool closes to free the bank.
        with (
            tc.tile_pool(name="warm", bufs=1) as warmp,
            tc.tile_pool(name="warmps", bufs=1, space="PSUM") as warmpsp,
        ):
            wt = warmp.tile([P, P], IN_DT, name="wt")
            nc.vector.memset(wt[:], 0.0)
            wps = warmpsp.tile([P, P], F32, name="wps")
            for _ in range(N_WARM):
                nc.tensor.matmul(wps[:], wt[:], wt[:], start=True, stop=True)

        misc = outer.enter_context(tc.tile_pool(name="misc", bufs=1))
        kvpool = outer.enter_context(tc.tile_pool(name="kvsb", bufs=1))
        # qt pool lives at OUTER scope so its SBUF region is disjoint from
        # the phase-A pools: the first chunks' qt DMAs (ring-ahead on the
        # gpsimd queue) then stream in during phase A with no anti-deps.
        qtp = outer.enter_context(tc.tile_pool(name="qt", bufs=2))
        wqp = outer.enter_context(tc.tile_pool(name="wqe", bufs=1))
        qm_sb = misc.tile([P, qcb], F32)
        km_sb = misc.tile([P, kc], F32)
        nc.scalar.dma_start(qm_sb[:], qm[:])
        nc.scalar.dma_start(km_sb[:], km[:])
        kv_sb = kvpool.tile([P, KV_W], num_dt)

        # ------ Phase A: K/V projection + kv accumulation ------------------
        # One pass, all 8 heads. Each kt d-chunk slice serves 4 consecutive
        # matmuls (K and V, two 512-wide o-halves each).
        esA = ExitStack()
        wkvp = esA.enter_context(tc.tile_pool(name="wkv", bufs=1))
        ktp = esA.enter_context(tc.tile_pool(name="kt", bufs=3))
        pkp = esA.enter_context(tc.tile_pool(name="pk", bufs=3))
        vap = esA.enter_context(tc.tile_pool(name="vaug", bufs=3))
        pps = esA.enter_context(tc.tile_pool(name="projps", bufs=5, space="PSUM"))
        kvps = esA.enter_context(tc.tile_pool(name="kvps", bufs=1, space="PSUM"))
        kv_ps = kvps.tile([P, 1536], F32)

        # kt chunk 0 FIRST on the sync ring so the opening matmuls wait on
        # 0.5 MB; weights stream concurrently on the gpsimd + scalar rings.
        kt0 = ktp.tile([P, D], IN_DT, tag="kt", name="kt0")
        nc.sync.dma_start(kt0[:], kT[:, 0:D])

        wk_sb = [
            wkvp.tile([P, OW], IN_DT, tag=f"wk{dc}", name=f"wk{dc}")
            for dc in range(DC)
        ]
        wv_sb = [
            wkvp.tile([P, OW], IN_DT, tag=f"wv{dc}", name=f"wv{dc}")
            for dc in range(DC)
        ]
        for dc in range(DC):
            nc.gpsimd.dma_start(wk_sb[dc][:], wk[dc * P : (dc + 1) * P, :])
            nc.scalar.dma_start(wv_sb[dc][:], wv[dc * P : (dc + 1) * P, :])
        # wq tiles are declared now but their DMAs are deferred until the
        # opening chunks have consumed wk/wv: the DMA engines are a shared
        # ~358 GB/s pool, and letting the (not-yet-needed) wq/qt stream run
        # early starves the critical phase-A weight loads.
        wq_sb = [
            wqp.tile([P, OW], IN_DT, tag=f"wq{dc}", name=f"wq{dc}")
            for dc in range(DC)
        ]

        bank_start = {}

        def emit_kv_mms(c, pk_sb, va_sb):
            for h in range(HL):
                bank_first = h % 3 == 0
                mm = nc.tensor.matmul(
                    kv_ps[:, _KV_BASE[h] : _KV_BASE[h] + 129],
                    pk_sb[:, h * P : (h + 1) * P],
                    va_sb[:, h * 129 : (h + 1) * 129],
                    start=(c == 0 and bank_first),
                    stop=(c == kc - 1),
                    skip_group_check=True,
                )
                if c == 0:
                    # start=True clears has_written for the whole PSUM bank;
                    # siblings must come after their bank's clear.
                    if bank_first:
                        bank_start[h // 3] = mm
                    else:
                        tile.add_dep_helper(
                            mm.ins,
                            bank_start[h // 3].ins,
                            reason="kv bank has_written clear order",
                        )

        # kv matmuls for chunk c are emitted after chunk c+1's projection
        # matmuls: their pk operand is only ready ~3us after chunk c's last
        # projection, so this keeps PE fed meanwhile.
        pending = None
        pps_allocs = 0
        wq_anchor = None
        wq_emitted = False
        for c in range(kc):
            if c == 0:
                kt_sb = kt0
            else:
                kt_sb = ktp.tile([P, D], IN_DT, tag="kt", name=f"kt{c}")
                nc.sync.dma_start(kt_sb[:], kT[:, c * D : (c + 1) * D])
            if not wq_emitted and wq_anchor is not None:
                # release the deferred wq stream (qt chunks ride behind it
                # on the same in-order gpsimd ring)
                dma = nc.gpsimd.dma_start(wq_sb[0][:], wq[0:P, :])
                tile.add_dep_helper(
                    dma.ins,
                    wq_anchor.ins,
                    sync=True,
                    reason="defer wq behind wk/wv",
                )
                for dc in range(1, DC):
                    nc.gpsimd.dma_start(
                        wq_sb[dc][:], wq[dc * P : (dc + 1) * P, :]
                    )
                wq_emitted = True
            if c == kc - 1:
                # rotate the proj-PSUM allocation so the final chunk's tiles
                # land on slots 1..4 and slot 0 (which phase B's first qp
                # tile will alias) was last touched two chunks earlier —
                # phase B's opening matmul then has no WAR stall.
                for _ in range((1 - pps_allocs) % 5):
                    pps.tile([P, 512], F32, tag="proj", name="spacer")
                    pps_allocs += 1
            kp0 = pps.tile([P, 512], F32, tag="proj", name="kp0")
            kp1 = pps.tile([P, 512], F32, tag="proj", name="kp1")
            vp0 = pps.tile([P, 512], F32, tag="proj", name="vp0")
            vp1 = pps.tile([P, 512], F32, tag="proj", name="vp1")
            pps_allocs += 4
            for dc in range(DC):
                lhsT = kt_sb[:, dc * P : (dc + 1) * P]
                st = dict(start=(dc == 0), stop=(dc == DC - 1))
                nc.tensor.matmul(kp0[:], lhsT, wk_sb[dc][:, 0:512], **st)
                nc.tensor.matmul(kp1[:], lhsT, wk_sb[dc][:, 512:1024], **st)
                nc.tensor.matmul(vp0[:], lhsT, wv_sb[dc][:, 0:512], **st)
                mmv = nc.tensor.matmul(vp1[:], lhsT, wv_sb[dc][:, 512:1024], **st)
                if c == min(1, kc - 1) and dc == DC - 1:
                    wq_anchor = mmv

            if pending is not None:
                emit_kv_mms(*pending)

            pk_sb = pkp.tile([P, OW], att_dt, tag="pk", name="pk")
            for half, kp in ((0, kp0), (1, kp1)):
                sa = pkp.tile([P, 512], F32, tag="sa", name="sa")
                sb = pkp.tile([P, 512], F32, tag="sb", name="sb")
                nc.scalar.activation(sa[:], kp[:], ABS)
                nc.scalar.activation(sb[:], sa[:], EXP, scale=-1.0)
                nc.scalar.activation(sa[:], sb[:], LN, bias=1.0)
                nc.vector.scalar_tensor_tensor(
                    pk_sb[:, half * 512 : (half + 1) * 512],
                    kp[:],
                    0.0,
                    sa[:],
                    MAX,
                    ADD,
                )
            nc.vector.tensor_scalar_mul(pk_sb[:], pk_sb[:], km_sb[:, c : c + 1])

            va_sb = vap.tile([P, HL * 129], att_dt, tag="vaug", name="va")
            nc.vector.memset(
                va_sb[:].rearrange("p (h x) -> p h x", x=129)[:, :, 128:129], 1.0
            )
            for h in range(HL):
                src = vp0 if h < 4 else vp1
                off = (h % 4) * P
                nc.vector.tensor_copy(
                    va_sb[:, h * 129 : h * 129 + P], src[:, off : off + P]
                )
            pending = (c, pk_sb, va_sb)

        if not wq_emitted:
            dma = nc.gpsimd.dma_start(wq_sb[0][:], wq[0:P, :])
            tile.add_dep_helper(
                dma.ins, wq_anchor.ins, sync=True, reason="defer wq behind wk/wv"
            )
            for dc in range(1, DC):
                nc.gpsimd.dma_start(wq_sb[dc][:], wq[dc * P : (dc + 1) * P, :])

        # Dependency-free bridge matmuls: the final chunk's kv matmuls wait
        # ~2.5us for its softplus, and the in-order PE queue would idle.
        # These land on proj-PSUM slot 0 (free since two chunks ago, thanks
        # to the spacer rotation) and read the resident kt tile, so they
        # issue immediately and keep the PE busy+warm across the boundary.
        bridge = pps.tile([P, 512], F32, tag="proj", name="bridge")
        pps_allocs += 1
        for _ in range(14):
            nc.tensor.matmul(
                bridge[:], kt_sb[:, 0:P], kt_sb[:, 0:512], start=True, stop=True
            )
        emit_kv_mms(*pending)

        for h in range(HL):
            nc.vector.tensor_copy(
                kv_sb[:, h * 129 : (h + 1) * 129],
                kv_ps[:, _KV_BASE[h] : _KV_BASE[h] + 129],
            )
        if KV_W > HL * 129:
            # f32r-typed zero pad (junk columns read by head 7's widened MM);
            # written by a DVE op so the FP32R-rounding verifier is satisfied.
            nc.vector.tensor_scalar_mul(
                kv_sb[:, HL * 129 : KV_W], kv_sb[:, 0:129], 0.0
            )
        esA.close()

        # ------ Phase B: Q projection + attention epilogue -----------------
        with (
            tc.tile_pool(name="pq", bufs=4) as pqp,
            tc.tile_pool(name="sc", bufs=4) as scp,
            tc.tile_pool(name="st", bufs=3) as stp,
            tc.tile_pool(name="qpps", bufs=4, space="PSUM") as qpps,
            tc.tile_pool(name="nmps", bufs=4, space="PSUM") as nmps,
        ):
            st_tiles = {}

            def emit_num(ci, o, W, h, pq_sb):
                # results stage into st (partition=l%128, cols j*OW+o) and
                # ship as ONE multi-run DMA per chunk — per-(h,j) output
                # DMAs cost ~0.6us of descriptor generation each.
                nj = W // P
                if h == 0:
                    st_tiles[ci] = stp.tile(
                        [P, nj * OW], OUT_DT, tag="st", name="st"
                    )
                st = st_tiles[ci]
                for j in range(nj):
                    nm = nmps.tile([P, NW], F32, tag="nm", name="nm")
                    nc.tensor.matmul(
                        nm[:],
                        pq_sb[:, j * P : (j + 1) * P],
                        kv_sb[:, h * 129 : h * 129 + NW],
                        start=True,
                        stop=True,
                    )
                    sc = scp.tile([P, 1], F32, tag="sc", name="sc")
                    nc.vector.reciprocal(sc[:], nm[:, 128:129])
                    col = o // P + j
                    nc.vector.tensor_scalar(
                        st[:, j * OW + h * P : j * OW + (h + 1) * P],
                        nm[:, 0:P],
                        sc[:, 0:1],
                        qm_sb[:, col : col + 1],
                        MUL,
                        MUL,
                    )
                if h == HL - 1:
                    nc.sync.dma_start(
                        out[o : o + W, :].rearrange("(j p) o -> p j o", p=P),
                        st[:].rearrange("p (j o) -> p j o", o=OW),
                    )
                    del st_tiles[ci]

            # num matmuls for step (ci,h) are emitted two steps behind the
            # projection matmuls (pq is ~2.5us of ACT/DVE behind qp; depth-2
            # keeps the PE fed through the boundary where the ACT queue is
            # still draining phase A's last softplus).
            from collections import deque

            pendq = deque()
            o = 0
            for ci, W in enumerate(qchunks):
                qt_sb = qtp.tile([P, DC * W], IN_DT, tag="qt", name=f"qt{ci}")
                nc.gpsimd.dma_start(
                    qt_sb[:], qT[:, DC * o : DC * (o + W)]
                )
                for h in range(HL):
                    qp = qpps.tile([P, W], F32, tag="qp", name="qp")
                    for dc in range(DC):
                        nc.tensor.matmul(
                            qp[:],
                            wq_sb[dc][:, h * P : (h + 1) * P],
                            qt_sb[:, dc * W : (dc + 1) * W],
                            start=(dc == 0),
                            stop=(dc == DC - 1),
                        )
                    if len(pendq) >= 2:
                        emit_num(*pendq.popleft())
                    pq_sb = pqp.tile([P, W], num_dt, tag="pq", name="pq")
                    sa = pqp.tile([P, W], F32, tag="sqa", name="sqa")
                    sb = pqp.tile([P, W], F32, tag="sqb", name="sqb")
                    nc.scalar.activation(sa[:], qp[:], ABS)
                    nc.scalar.activation(sb[:], sa[:], EXP, scale=-1.0)
                    nc.scalar.activation(sa[:], sb[:], LN, bias=1.0)
                    nc.vector.scalar_tensor_tensor(
                        pq_sb[:], qp[:], 0.0, sa[:], MAX, ADD
                    )
                    pendq.append((ci, o, W, h, pq_sb))
                o += W
            while pendq:
                emit_num(*pendq.popleft())
    return nc


def _get_nc(kc: int, qchunks: tuple) -> bass.Bass:
    key = (PREC, kc, qchunks)
    if key not in _CACHED_NC:
        _CACHED_NC[key] = _build_nc(kc, qchunks)
    return _CACHED_NC[key]


def _qchunk_widths(qtot: int) -> tuple:
    """Split qtot (a multiple of 128) into 512-wide chunks plus at most one
    smaller remainder chunk, remainder LAST so the kernel tail is short."""
    nfull, rem = divmod(qtot, 512)
    w = [512] * nfull
    if rem:
        w.append(rem)
    return tuple(w)


def kernel(query, key, Wq, Wk, Wv, query_padding_mask, key_padding_mask):
    global LAST_EXEC_TIME_NS
    query = np.asarray(query, dtype=np.float32)
    key = np.asarray(key, dtype=np.float32)
    Wq = np.asarray(Wq, dtype=np.float32)
    Wk = np.asarray(Wk, dtype=np.float32)
    Wv = np.asarray(Wv, dtype=np.float32)
    qmask = np.asarray(query_padding_mask)
    kmask = np.asarray(key_padding_mask)

    in_dt = np.float32
    if PREC == "fp16":
        in_dt = np.float16
    elif PREC == "bf16":
        import ml_dtypes

        in_dt = ml_dtypes.bfloat16

    # Compaction: gather unpadded rows, pad to a common (over batches)
    # multiple of 128. Padded rows are zeros with keep=0 so they contribute
    # exactly nothing; query rows are scattered back on host.
    kidxs = [np.flatnonzero(~kmask[n]) for n in range(N)]
    qidxs = [np.flatnonzero(~qmask[n]) for n in range(N)]
    kmax = max(max(len(ix) for ix in kidxs), 1)
    qmax = max(max(len(ix) for ix in qidxs), 1)
    kc = -(-kmax // P)  # key chunks of 128
    ktot = kc * P
    qtot = -(-qmax // P) * P
    qchunks = _qchunk_widths(qtot)

    nc = _get_nc(kc, qchunks)

    in_maps = []
    for c in range(NCORES):
        n, g = c // 2, c % 2
        sl = slice(g * OW, (g + 1) * OW)
        kidx, qidx = kidxs[n], qidxs[n]

        key_c = np.zeros((ktot, D), np.float32)
        key_c[: len(kidx)] = key[n][kidx]
        # kT packing: [p, c*D + dc*128 + j] = key_c[c*128 + j, dc*128 + p]
        kT2 = np.ascontiguousarray(
            key_c.reshape(kc, P, DC, P).transpose(3, 0, 2, 1).reshape(P, kc * D)
        ).astype(in_dt)
        km2 = np.zeros(ktot, np.float32)
        km2[: len(kidx)] = 1.0
        km2 = np.ascontiguousarray(km2.reshape(kc, P).T)

        query_c = np.zeros((qtot, D), np.float32)
        query_c[: len(qidx)] = query[n][qidx]
        # qT packing per chunk: [p, dc*W + j] = query_c[o + j, dc*128 + p]
        blocks = []
        o = 0
        for W in qchunks:
            blocks.append(
                query_c[o : o + W].reshape(W, DC, P).transpose(2, 1, 0).reshape(P, DC * W)
            )
            o += W
        qT2 = np.ascontiguousarray(np.concatenate(blocks, axis=1)).astype(in_dt)
        qm2 = np.zeros(qtot, np.float32)
        qm2[: len(qidx)] = 1.0
        qm2 = np.ascontiguousarray(qm2.reshape(-1, P).T)

        in_maps.append(
            {
                "qT": qT2,
                "kT": kT2,
                "wq": np.ascontiguousarray(Wq[sl, :].T.astype(in_dt)),
                "wk": np.ascontiguousarray(Wk[sl, :].T.astype(in_dt)),
                "wv": np.ascontiguousarray(Wv[sl, :].T.astype(in_dt)),
                "qm": qm2,
                "km": km2,
            }
        )

    res = bu.run_bass_kernel_spmd(
        nc, in_maps, core_ids=list(range(NCORES)), trace=TRACE
    )
    LAST_EXEC_TIME_NS = res.exec_time_ns

    full = np.zeros((N, L, D), dtype=np.float32)
    for c in range(NCORES):
        n, g = c // 2, c % 2
        qidx = qidxs[n]
        full[n, qidx, g * OW : (g + 1) * OW] = res.results[c]["out"][: len(qidx)]
    return full


# revision 21
# speedup vs baseline: 1.1111x; 1.0031x over previous
"""Trainium2 Bass kernel for nn_MultiHeadAttention_89429809037632.

Linear attention (softplus feature map) with padding masks:
    q = query @ Wq.T ; k = key @ Wk.T ; v = key @ Wv.T   (per-head split)
    pq = softplus(q) ; pk = softplus(k) * keep(key_mask)
    kv = pk^T v (per head, plus a fused ones-column giving sum(pk))
    out = (pq @ kv) / (pq @ sum(pk)) * keep(query_mask)

Sharding across 8 NeuronCores: data-parallel over N=4 batches x
tensor-parallel over 2 head-groups (8 heads x 128 dims = 1024 output
dims each). Host transposes activations/weights so the contraction
dim (D) lands on the SBUF partition axis; each core runs an identical
SPMD program on its shard, outputs are concatenated on host.

Padding compaction: ~10% of keys/queries are padded (masked to zero
contribution / zero output). The host gathers the unpadded rows,
pads to a common 128-multiple across batches (compiled shapes depend
only on the rounded counts, cached), and scatters the output back.
Removed keys contribute exactly 0 (pk=0) so numerics are unchanged;
this cuts ~9% of all three projection GEMMs (PE issue time is the
bottleneck at 94% tensor-engine occupancy).

Per-core program (Tile framework), fp16 inputs with fp32 PSUM
accumulation (measured ~4.4e-4 scale-relative absmax):
  Warmup: ~96 dependency-free dummy matmuls issued at t=0 keep the PE
    busy while the first DMAs land, flipping the HAM clock gate from
    1.2GHz to 2.4GHz before real work starts.
  Phase A: for each 128-key chunk: project K,V (full-rate matmuls,
    stationary = key^T d-chunk serving 4 matmuls), softplus+mask -> pk,
    copy V into a [v | 1] block layout, then 8 per-head matmuls
    accumulate kv_aug (128x129 per head) in PSUM across all chunks.
  Phase B: for each query chunk (512-query chunks + one remainder) x
    head: project Q, softplus -> pq, then per 128-query subchunk one
    matmul against kv_aug gives [num | den]; epilogue computes
    num * (keep/den) on DVE into a per-chunk staging tile shipped as
    one chunk-sized DMA.
  kT/qT are host-packed so each chunk tile loads with ONE dma_start of
  4-16KB contiguous per-partition runs. DMA descriptor generation is
  spread over three rings (kt+out on sync, wk+wq+qt on gpsimd, wv on
  scalar) so the opening weight stream isn't serialized behind one
  sequencer. Matmul emission is software-pipelined (kv/num matmuls
  trail their producer chunk by one step).
"""

import json
import os
import sys
import types

import numpy as np

for _p in ("/opt/trn_rl_repo",):
    if _p not in sys.path and os.path.isdir(_p):
        sys.path.insert(0, _p)

# ``run_bass_kernel_spmd(trace=True)`` imports antenv.axon_hooks, which not
# every image ships. Provide a stub so the import never crashes (returning
# None simply disables NTFF tracing).
try:
    import antenv.axon_hooks  # noqa: F401
except Exception:
    try:
        import antenv

        _m = types.ModuleType("antenv.axon_hooks")
        _HOOK = [None]

        def _get_hook():
            if _HOOK[0] is None:
                try:
                    from trn_agent_boot.trn_boot import _ntff_profile_via_ctypes

                    _HOOK[0] = _ntff_profile_via_ctypes("/opt/axon/libaxon_pjrt.so")
                except Exception:
                    _HOOK[0] = None
            return _HOOK[0]

        _m.get_axon_ntff_profile_hook = _get_hook
        _m.set_axon_ntff_profile_hook = lambda h: _HOOK.__setitem__(0, h)
        sys.modules["antenv.axon_hooks"] = _m
        antenv.axon_hooks = _m
    except Exception:
        pass

import concourse.bass as bass
import concourse.bass_utils as bu
import concourse.mybir as mybir
import concourse.tile as tile

# ---------------------------------------------------------------------------
# Shim 1: this container's walrus accepts only ONE sync-wait per instruction
# ("Too many sync wait commands"); Tile attaches several. Rewrite the BIR
# JSON so excess waits ride on same-engine NoOps immediately before the
# instruction (engine streams are in-order, so this is equivalent).
# Shim 2: upload_artifacts wants a cloud bucket; keep artifacts local.
# ---------------------------------------------------------------------------
_MAX_WAITS = 1


def _split_multi_waits(bir_bytes: bytes) -> bytes:
    d = json.loads(bir_bytes)
    ctr = 0
    changed = False
    for fn in d.get("functions", []):
        for bb in fn.get("blocks", []):
            out = []
            for inst in bb.get("instructions", []):
                si = inst.get("sync_info")
                waits = (si or {}).get("on_wait") or []
                if len(waits) > _MAX_WAITS:
                    changed = True
                    idx = 0
                    while len(waits) - idx > _MAX_WAITS:
                        chunk = waits[idx : idx + _MAX_WAITS]
                        idx += _MAX_WAITS
                        ctr += 1
                        nop = {
                            "engine": inst["engine"],
                            "ins": [],
                            "outs": [],
                            "name": f"I-wsplit-{ctr}",
                            "opcode": "NoOp",
                            "sync_info": {"on_update": [], "on_wait": chunk},
                        }
                        if "debug" in inst:
                            nop["debug"] = inst["debug"]
                        out.append(nop)
                    si["on_wait"] = waits[idx:]
                out.append(inst)
            bb["instructions"] = out
    return json.dumps(d).encode() if changed else bir_bytes


if not getattr(bass.Bass, "_wait_split_shim", False):
    _orig_to_json = bass.Bass.to_json_bytes

    def _to_json_bytes(self) -> bytes:
        return _split_multi_waits(_orig_to_json(self))

    bass.Bass.to_json_bytes = _to_json_bytes
    bass.Bass._wait_split_shim = True
    bu.upload_artifacts = lambda tmpdir: tmpdir

# ---------------------------------------------------------------------------
# Problem shapes (hardcoded per contract)
# ---------------------------------------------------------------------------
N, L, D = 4, 4096, 2048  # batches, seq len (q and k), model dim
H, P = 16, 128  # heads, head dim
NCORES = 8
HL = H // 2  # heads per core (head-group of 8)
OW = HL * P  # per-core projected width (1024)
DC = D // P  # 16 contraction chunks

F32 = mybir.dt.float32
F32R = mybir.dt.float32r
# The ACT tables in this walrus build ship no softplus; synthesize the
# numerically stable form softplus(x) = max(x,0) + ln(1 + exp(-|x|)) from
# set 6 ("natural_log_exp_and_others": abs/exp/ln in one resident table).
ABS = mybir.ActivationFunctionType.Abs
EXP = mybir.ActivationFunctionType.Exp
LN = mybir.ActivationFunctionType.Ln
MUL = mybir.AluOpType.mult
MAX = mybir.AluOpType.max
ADD = mybir.AluOpType.add

# kv_aug per-head column offsets inside the 3-bank PSUM accumulator:
# 3 heads per 2 KiB bank (129 fp32 columns each, none crossing a bank edge).
_KV_BASE = [(h // 3) * 512 + (h % 3) * 129 for h in range(HL)]

TRACE = False  # set True (e.g. from test.py) to capture NTFF profile
LAST_EXEC_TIME_NS = None

# Precision mode for all matmuls:
#   "f32r": projections in float32r (full rate, ~1.6e-4 end rel err),
#           attention stage fp32 (1/4-rate small matmuls, widened f32r num).
#   "fp16": everything fp16 inputs + fp32 PSUM accumulation (full rate,
#           ~11-bit input rounding => ~5e-4 end rel err, fastest).
#   "bf16": like fp16 but 8-bit mantissa (~2e-3 end rel err).
PREC = "fp16"

N_WARM = 96  # dummy matmuls to warm the HAM clock gate while DMAs land

_CACHED_NC = {}


def _build_nc(kc: int, qchunks: tuple) -> bass.Bass:
    """Build the per-core program for kc 128-key chunks and the given
    query chunk widths (each a multiple of 128; 512 in steady state)."""
    from contextlib import ExitStack

    qtot = sum(qchunks)
    qcb = qtot // P  # query keep-mask columns

    lp = {"fp16": mybir.dt.float16, "bf16": mybir.dt.bfloat16}.get(PREC)
    IN_DT = F32R if lp is None else lp  # projections (dram + sbuf operands)
    att_dt = F32 if lp is None else lp  # kv-stage operands (pk, v_aug)
    # num-stage matmul: fp16/bf16 run full-rate at any width; the f32r path
    # widens to 258 moving columns (>=256 streams at 1 cyc/row vs fp32's 4)
    # — the upper 129 columns are discarded junk.
    num_dt = F32R if lp is None else lp
    NW = 258 if lp is None else 129
    KV_W = HL * 129 + (129 if lp is None else 0)  # pad so h=7 reads 258 cols
    # fp16 output staging+DMA (host converts back): halves the output
    # traffic and the kernel-tail DMA; adds ~2.4e-4 relative rounding.
    OUT_DT = F32 if lp is None else mybir.dt.float16

    nc = bass.Bass()
    # qT/kT are host-packed: column block c holds that chunk's activations
    # with layout [p, dc*W + j] = act[chunk_off + j, dc*128 + p], so a chunk
    # loads as ONE dma_start with a contiguous per-partition run and the
    # dc-slices come out as plain column slices.
    qT = nc.dram_tensor("qT", (P, DC * qtot), IN_DT, kind="ExternalInput")
    kT = nc.dram_tensor("kT", (P, kc * D), IN_DT, kind="ExternalInput")
    wq = nc.dram_tensor("wq", (D, OW), IN_DT, kind="ExternalInput")
    wk = nc.dram_tensor("wk", (D, OW), IN_DT, kind="ExternalInput")
    wv = nc.dram_tensor("wv", (D, OW), IN_DT, kind="ExternalInput")
    qm = nc.dram_tensor("qm", (P, qcb), F32, kind="ExternalInput")
    km = nc.dram_tensor("km", (P, kc), F32, kind="ExternalInput")
    out = nc.dram_tensor("out", (qtot, OW), OUT_DT, kind="ExternalOutput")

    with tile.TileContext(nc) as tc, ExitStack() as outer:
        # ------ PE warmup: the HAM clock gate starts at 1.2GHz and needs
        # ~3.4us of sustained activity to release to 2.4GHz. The opening
        # DMAs take ~10us to land; fill that window with dependency-free
        # matmuls so real work starts warm. Pool closes to free the bank.
        with (
            tc.tile_pool(name="warm", bufs=1) as warmp,
            tc.tile_pool(name="warmps", bufs=1, space="PSUM") as warmpsp,
        ):
            wt = warmp.tile([P, P], IN_DT, name="wt")
            nc.gpsimd.memset(wt[:], 0.0)
            wps = warmpsp.tile([P, P], F32, name="wps")
            for _ in range(N_WARM):
                nc.tensor.matmul(wps[:], wt[:], wt[:], start=True, stop=True)

        misc = outer.enter_context(tc.tile_pool(name="misc", bufs=1))
        kvpool = outer.enter_context(tc.tile_pool(name="kvsb", bufs=1))
        # qt pool lives at OUTER scope so its SBUF region is disjoint from
        # the phase-A pools: the first chunks' qt DMAs (ring-ahead on the
        # gpsimd queue) then stream in during phase A with no anti-deps.
        qtp = outer.enter_context(tc.tile_pool(name="qt", bufs=2))
        wqp = outer.enter_context(tc.tile_pool(name="wqe", bufs=1))
        qm_sb = misc.tile([P, qcb], F32)
        km_sb = misc.tile([P, kc], F32)
        nc.scalar.dma_start(qm_sb[:], qm[:])
        nc.scalar.dma_start(km_sb[:], km[:])
        kv_sb = kvpool.tile([P, KV_W], num_dt)

        # ------ Phase A: K/V projection + kv accumulation ------------------
        # One pass, all 8 heads. Each kt d-chunk slice serves 4 consecutive
        # matmuls (K and V, two 512-wide o-halves each).
        esA = ExitStack()
        wkvp = esA.enter_context(tc.tile_pool(name="wkv", bufs=1))
        ktp = esA.enter_context(tc.tile_pool(name="kt", bufs=3))
        pkp = esA.enter_context(tc.tile_pool(name="pk", bufs=3))
        vap = esA.enter_context(tc.tile_pool(name="vaug", bufs=3))
        pps = esA.enter_context(tc.tile_pool(name="projps", bufs=5, space="PSUM"))
        kvps = esA.enter_context(tc.tile_pool(name="kvps", bufs=1, space="PSUM"))
        kv_ps = kvps.tile([P, 1536], F32)

        # kt chunk 0 FIRST on the sync ring so the opening matmuls wait on
        # 0.5 MB; weights stream concurrently on the gpsimd + scalar rings.
        kt0 = ktp.tile([P, D], IN_DT, tag="kt", name="kt0")
        nc.sync.dma_start(kt0[:], kT[:, 0:D])

        wk_sb = [
            wkvp.tile([P, OW], IN_DT, tag=f"wk{dc}", name=f"wk{dc}")
            for dc in range(DC)
        ]
        wv_sb = [
            wkvp.tile([P, OW], IN_DT, tag=f"wv{dc}", name=f"wv{dc}")
            for dc in range(DC)
        ]
        # wk/wv interleaved on ONE ring: FIFO delivery matches the per-dc
        # consumption order exactly (the per-HWDGE scalar ring measured only
        # ~110 GB/s and made wv the straggler when split across rings).
        for dc in range(DC):
            nc.gpsimd.dma_start(wk_sb[dc][:], wk[dc * P : (dc + 1) * P, :])
            nc.gpsimd.dma_start(wv_sb[dc][:], wv[dc * P : (dc + 1) * P, :])
        # wq tiles are declared now but their DMAs are deferred until the
        # opening chunks have consumed wk/wv: the DMA engines are a shared
        # ~358 GB/s pool, and letting the (not-yet-needed) wq/qt stream run
        # early starves the critical phase-A weight loads.
        wq_sb = [
            wqp.tile([P, OW], IN_DT, tag=f"wq{dc}", name=f"wq{dc}")
            for dc in range(DC)
        ]

        bank_start = {}

        def emit_kv_mms(c, pk_sb, va_sb):
            for h in range(HL):
                bank_first = h % 3 == 0
                mm = nc.tensor.matmul(
                    kv_ps[:, _KV_BASE[h] : _KV_BASE[h] + 129],
                    pk_sb[:, h * P : (h + 1) * P],
                    va_sb[:, h * 129 : (h + 1) * 129],
                    start=(c == 0 and bank_first),
                    stop=(c == kc - 1),
                    skip_group_check=True,
                )
                if c == 0:
                    # start=True clears has_written for the whole PSUM bank;
                    # siblings must come after their bank's clear.
                    if bank_first:
                        bank_start[h // 3] = mm
                    else:
                        tile.add_dep_helper(
                            mm.ins,
                            bank_start[h // 3].ins,
                            reason="kv bank has_written clear order",
                        )

        # kv matmuls for chunk c are emitted after chunk c+1's projection
        # matmuls: their pk operand is only ready ~3us after chunk c's last
        # projection, so this keeps PE fed meanwhile.
        pending = None
        pps_allocs = 0
        wq_anchor = None
        wq_emitted = False
        for c in range(kc):
            if c == 0:
                kt_sb = kt0
            else:
                kt_sb = ktp.tile([P, D], IN_DT, tag="kt", name=f"kt{c}")
                nc.sync.dma_start(kt_sb[:], kT[:, c * D : (c + 1) * D])
            if not wq_emitted and wq_anchor is not None:
                # release the deferred wq stream (qt chunks ride behind it
                # on the same in-order gpsimd ring)
                dma = nc.scalar.dma_start(wq_sb[0][:], wq[0:P, :])
                tile.add_dep_helper(
                    dma.ins,
                    wq_anchor.ins,
                    sync=True,
                    reason="defer wq behind wk/wv",
                )
                for dc in range(1, DC):
                    nc.scalar.dma_start(
                        wq_sb[dc][:], wq[dc * P : (dc + 1) * P, :]
                    )
                wq_emitted = True
            if c == kc - 1:
                # rotate the proj-PSUM allocation so the final chunk's tiles
                # land on slots 1..4 and slot 0 (which phase B's first qp
                # tile will alias) was last touched two chunks earlier —
                # phase B's opening matmul then has no WAR stall.
                for _ in range((1 - pps_allocs) % 5):
                    pps.tile([P, 512], F32, tag="proj", name="spacer")
                    pps_allocs += 1
            kp0 = pps.tile([P, 512], F32, tag="proj", name="kp0")
            kp1 = pps.tile([P, 512], F32, tag="proj", name="kp1")
            vp0 = pps.tile([P, 512], F32, tag="proj", name="vp0")
            vp1 = pps.tile([P, 512], F32, tag="proj", name="vp1")
            pps_allocs += 4
            for dc in range(DC):
                lhsT = kt_sb[:, dc * P : (dc + 1) * P]
                st = dict(start=(dc == 0), stop=(dc == DC - 1))
                nc.tensor.matmul(kp0[:], lhsT, wk_sb[dc][:, 0:512], **st)
                nc.tensor.matmul(kp1[:], lhsT, wk_sb[dc][:, 512:1024], **st)
                nc.tensor.matmul(vp0[:], lhsT, wv_sb[dc][:, 0:512], **st)
                mmv = nc.tensor.matmul(vp1[:], lhsT, wv_sb[dc][:, 512:1024], **st)
                if c == min(1, kc - 1) and dc == DC - 1:
                    wq_anchor = mmv

            if pending is not None:
                emit_kv_mms(*pending)

            pk_sb = pkp.tile([P, OW], att_dt, tag="pk", name="pk")
            for half, kp in ((0, kp0), (1, kp1)):
                sa = pkp.tile([P, 512], F32, tag="sa", name="sa")
                sb = pkp.tile([P, 512], F32, tag="sb", name="sb")
                nc.scalar.activation(sa[:], kp[:], ABS)
                nc.scalar.activation(sb[:], sa[:], EXP, scale=-1.0)
                nc.scalar.activation(sa[:], sb[:], LN, bias=1.0)
                nc.vector.scalar_tensor_tensor(
                    pk_sb[:, half * 512 : (half + 1) * 512],
                    kp[:],
                    0.0,
                    sa[:],
                    MAX,
                    ADD,
                )
            nc.vector.tensor_scalar_mul(pk_sb[:], pk_sb[:], km_sb[:, c : c + 1])

            va_sb = vap.tile([P, HL * 129], att_dt, tag="vaug", name="va")
            nc.vector.memset(
                va_sb[:].rearrange("p (h x) -> p h x", x=129)[:, :, 128:129], 1.0
            )
            for h in range(HL):
                src = vp0 if h < 4 else vp1
                off = (h % 4) * P
                nc.vector.tensor_copy(
                    va_sb[:, h * 129 : h * 129 + P], src[:, off : off + P]
                )
            pending = (c, pk_sb, va_sb)

        if not wq_emitted:
            dma = nc.scalar.dma_start(wq_sb[0][:], wq[0:P, :])
            tile.add_dep_helper(
                dma.ins, wq_anchor.ins, sync=True, reason="defer wq behind wk/wv"
            )
            for dc in range(1, DC):
                nc.scalar.dma_start(wq_sb[dc][:], wq[dc * P : (dc + 1) * P, :])

        # Dependency-free bridge matmuls: the final chunk's kv matmuls wait
        # ~2.5us for its softplus, and the in-order PE queue would idle.
        # These land on proj-PSUM slot 0 (free since two chunks ago, thanks
        # to the spacer rotation) and read the resident kt tile, so they
        # issue immediately and keep the PE busy+warm across the boundary.
        bridge = pps.tile([P, 512], F32, tag="proj", name="bridge")
        pps_allocs += 1
        for _ in range(14):
            nc.tensor.matmul(
                bridge[:], kt_sb[:, 0:P], kt_sb[:, 0:512], start=True, stop=True
            )
        emit_kv_mms(*pending)

        for h in range(HL):
            nc.vector.tensor_copy(
                kv_sb[:, h * 129 : (h + 1) * 129],
                kv_ps[:, _KV_BASE[h] : _KV_BASE[h] + 129],
            )
        if KV_W > HL * 129:
            # f32r-typed zero pad (junk columns read by head 7's widened MM);
            # written by a DVE op so the FP32R-rounding verifier is satisfied.
            nc.vector.tensor_scalar_mul(
                kv_sb[:, HL * 129 : KV_W], kv_sb[:, 0:129], 0.0
            )
        esA.close()

        # ------ Phase B: Q projection + attention epilogue -----------------
        with (
            tc.tile_pool(name="pq", bufs=4) as pqp,
            tc.tile_pool(name="sc", bufs=4) as scp,
            tc.tile_pool(name="st", bufs=3) as stp,
            tc.tile_pool(name="qpps", bufs=4, space="PSUM") as qpps,
            tc.tile_pool(name="nmps", bufs=4, space="PSUM") as nmps,
        ):
            st_tiles = {}

            def emit_num(ci, o, W, h, pq_sb):
                # results stage into st (partition=l%128, cols j*OW+o) and
                # ship as ONE multi-run DMA per chunk — per-(h,j) output
                # DMAs cost ~0.6us of descriptor generation each.
                nj = W // P
                if h == 0:
                    st_tiles[ci] = stp.tile(
                        [P, nj * OW], OUT_DT, tag="st", name="st"
                    )
                st = st_tiles[ci]
                for j in range(nj):
                    nm = nmps.tile([P, NW], F32, tag="nm", name="nm")
                    nc.tensor.matmul(
                        nm[:],
                        pq_sb[:, j * P : (j + 1) * P],
                        kv_sb[:, h * 129 : h * 129 + NW],
                        start=True,
                        stop=True,
                    )
                    sc = scp.tile([P, 1], F32, tag="sc", name="sc")
                    nc.vector.reciprocal(sc[:], nm[:, 128:129])
                    col = o // P + j
                    nc.vector.tensor_scalar(
                        st[:, j * OW + h * P : j * OW + (h + 1) * P],
                        nm[:, 0:P],
                        sc[:, 0:1],
                        qm_sb[:, col : col + 1],
                        MUL,
                        MUL,
                    )
                if h == HL - 1:
                    nc.sync.dma_start(
                        out[o : o + W, :].rearrange("(j p) o -> p j o", p=P),
                        st[:].rearrange("p (j o) -> p j o", o=OW),
                    )
                    del st_tiles[ci]

            # num matmuls for step (ci,h) are emitted two steps behind the
            # projection matmuls (pq is ~2.5us of ACT/DVE behind qp; depth-2
            # keeps the PE fed through the boundary where the ACT queue is
            # still draining phase A's last softplus).
            from collections import deque

            pendq = deque()
            o = 0
            for ci, W in enumerate(qchunks):
                qt_sb = qtp.tile([P, DC * W], IN_DT, tag="qt", name=f"qt{ci}")
                nc.gpsimd.dma_start(
                    qt_sb[:], qT[:, DC * o : DC * (o + W)]
                )
                for h in range(HL):
                    qp = qpps.tile([P, W], F32, tag="qp", name="qp")
                    for dc in range(DC):
                        nc.tensor.matmul(
                            qp[:],
                            wq_sb[dc][:, h * P : (h + 1) * P],
                            qt_sb[:, dc * W : (dc + 1) * W],
                            start=(dc == 0),
                            stop=(dc == DC - 1),
                        )
                    # depth-2 through the boundary; depth-1 on the last chunk
                    # so the kernel tail (drain with no projs left) is short
                    depth = 1 if ci == len(qchunks) - 1 else 2
                    while len(pendq) >= depth:
                        emit_num(*pendq.popleft())
                    pq_sb = pqp.tile([P, W], num_dt, tag="pq", name="pq")
                    sa = pqp.tile([P, W], F32, tag="sqa", name="sqa")
                    sb = pqp.tile([P, W], F32, tag="sqb", name="sqb")
                    nc.scalar.activation(sa[:], qp[:], ABS)
                    nc.scalar.activation(sb[:], sa[:], EXP, scale=-1.0)
                    nc.scalar.activation(sa[:], sb[:], LN, bias=1.0)
                    nc.vector.scalar_tensor_tensor(
                        pq_sb[:], qp[:], 0.0, sa[:], MAX, ADD
                    )
                    pendq.append((ci, o, W, h, pq_sb))
                o += W
            while pendq:
                emit_num(*pendq.popleft())
    return nc


def _get_nc(kc: int, qchunks: tuple) -> bass.Bass:
    key = (PREC, kc, qchunks)
    if key not in _CACHED_NC:
        _CACHED_NC[key] = _build_nc(kc, qchunks)
    return _CACHED_NC[key]


def _qchunk_widths(qtot: int) -> tuple:
    """Split qtot (a multiple of 128) into 512-wide chunks plus at most one
    smaller remainder chunk, remainder LAST so the kernel tail is short."""
    nfull, rem = divmod(qtot, 512)
    w = [512] * nfull
    if rem:
        w.append(rem)
    return tuple(w)


def kernel(query, key, Wq, Wk, Wv, query_padding_mask, key_padding_mask):
    global LAST_EXEC_TIME_NS
    query = np.asarray(query, dtype=np.float32)
    key = np.asarray(key, dtype=np.float32)
    Wq = np.asarray(Wq, dtype=np.float32)
    Wk = np.asarray(Wk, dtype=np.float32)
    Wv = np.asarray(Wv, dtype=np.float32)
    qmask = np.asarray(query_padding_mask)
    kmask = np.asarray(key_padding_mask)

    in_dt = np.float32
    if PREC == "fp16":
        in_dt = np.float16
    elif PREC == "bf16":
        import ml_dtypes

        in_dt = ml_dtypes.bfloat16

    # Compaction: gather unpadded rows, pad to a common (over batches)
    # multiple of 128. Padded rows are zeros with keep=0 so they contribute
    # exactly nothing; query rows are scattered back on host.
    kidxs = [np.flatnonzero(~kmask[n]) for n in range(N)]
    qidxs = [np.flatnonzero(~qmask[n]) for n in range(N)]
    kmax = max(max(len(ix) for ix in kidxs), 1)
    qmax = max(max(len(ix) for ix in qidxs), 1)
    kc = -(-kmax // P)  # key chunks of 128
    ktot = kc * P
    qtot = -(-qmax // P) * P
    qchunks = _qchunk_widths(qtot)

    nc = _get_nc(kc, qchunks)

    in_maps = []
    for c in range(NCORES):
        n, g = c // 2, c % 2
        sl = slice(g * OW, (g + 1) * OW)
        kidx, qidx = kidxs[n], qidxs[n]

        key_c = np.zeros((ktot, D), np.float32)
        key_c[: len(kidx)] = key[n][kidx]
        # kT packing: [p, c*D + dc*128 + j] = key_c[c*128 + j, dc*128 + p]
        kT2 = np.ascontiguousarray(
            key_c.reshape(kc, P, DC, P).transpose(3, 0, 2, 1).reshape(P, kc * D)
        ).astype(in_dt)
        km2 = np.zeros(ktot, np.float32)
        km2[: len(kidx)] = 1.0
        km2 = np.ascontiguousarray(km2.reshape(kc, P).T)

        query_c = np.zeros((qtot, D), np.float32)
        query_c[: len(qidx)] = query[n][qidx]
        # qT packing per chunk: [p, dc*W + j] = query_c[o + j, dc*128 + p]
        blocks = []
        o = 0
        for W in qchunks:
            blocks.append(
                query_c[o : o + W].reshape(W, DC, P).transpose(2, 1, 0).reshape(P, DC * W)
            )
            o += W
        qT2 = np.ascontiguousarray(np.concatenate(blocks, axis=1)).astype(in_dt)
        qm2 = np.zeros(qtot, np.float32)
        qm2[: len(qidx)] = 1.0
        qm2 = np.ascontiguousarray(qm2.reshape(-1, P).T)

        in_maps.append(
            {
                "qT": qT2,
                "kT": kT2,
                "wq": np.ascontiguousarray(Wq[sl, :].T.astype(in_dt)),
                "wk": np.ascontiguousarray(Wk[sl, :].T.astype(in_dt)),
                "wv": np.ascontiguousarray(Wv[sl, :].T.astype(in_dt)),
                "qm": qm2,
                "km": km2,
            }
        )

    res = bu.run_bass_kernel_spmd(
        nc, in_maps, core_ids=list(range(NCORES)), trace=TRACE
    )
    LAST_EXEC_TIME_NS = res.exec_time_ns

    full = np.zeros((N, L, D), dtype=np.float32)
    for c in range(NCORES):
        n, g = c // 2, c % 2
        qidx = qidxs[n]
        full[n, qidx, g * OW : (g + 1) * OW] = res.results[c]["out"][: len(qidx)]
    return full


# revision 29
# speedup vs baseline: 1.1288x; 1.0159x over previous
"""Trainium2 Bass kernel for nn_MultiHeadAttention_89429809037632.

Linear attention (softplus feature map) with padding masks:
    q = query @ Wq.T ; k = key @ Wk.T ; v = key @ Wv.T   (per-head split)
    pq = softplus(q) ; pk = softplus(k) * keep(key_mask)
    kv = pk^T v (per head, plus a fused ones-column giving sum(pk))
    out = (pq @ kv) / (pq @ sum(pk)) * keep(query_mask)

Sharding across 8 NeuronCores: data-parallel over N=4 batches x
tensor-parallel over 2 head-groups (8 heads x 128 dims = 1024 output
dims each). Host transposes activations/weights so the contraction
dim (D) lands on the SBUF partition axis; each core runs an identical
SPMD program on its shard, outputs are concatenated on host.

Padding compaction: ~10% of keys/queries are padded (masked to zero
contribution / zero output). The host gathers the unpadded rows,
pads to a common 128-multiple across batches (compiled shapes depend
only on the rounded counts, cached), and scatters the output back.
Removed keys contribute exactly 0 (pk=0) so numerics are unchanged;
this cuts ~9% of all three projection GEMMs (PE issue time is the
bottleneck at 94% tensor-engine occupancy).

Per-core program (Tile framework), fp16 inputs with fp32 PSUM
accumulation (measured ~4.4e-4 scale-relative absmax):
  Warmup: ~96 dependency-free dummy matmuls issued at t=0 keep the PE
    busy while the first DMAs land, flipping the HAM clock gate from
    1.2GHz to 2.4GHz before real work starts.
  Phase A: for each 128-key chunk: project K,V (full-rate matmuls,
    stationary = key^T d-chunk serving 4 matmuls), softplus+mask -> pk,
    copy V into a [v | 1] block layout, then 8 per-head matmuls
    accumulate kv_aug (128x129 per head) in PSUM across all chunks.
  Phase B: for each query chunk (512-query chunks + one remainder) x
    head: project Q, softplus -> pq, then per 128-query subchunk one
    matmul against kv_aug gives [num | den]; epilogue computes
    num * (keep/den) on DVE into a per-chunk staging tile shipped as
    one chunk-sized DMA.
  kT/qT are host-packed so each chunk tile loads with ONE dma_start of
  4-16KB contiguous per-partition runs. DMA descriptor generation is
  spread over three rings (kt+out on sync, wk+wq+qt on gpsimd, wv on
  scalar) so the opening weight stream isn't serialized behind one
  sequencer. Matmul emission is software-pipelined (kv/num matmuls
  trail their producer chunk by one step).
"""

import json
import os
import sys
import types

import numpy as np

for _p in ("/opt/trn_rl_repo",):
    if _p not in sys.path and os.path.isdir(_p):
        sys.path.insert(0, _p)

# ``run_bass_kernel_spmd(trace=True)`` imports antenv.axon_hooks, which not
# every image ships. Provide a stub so the import never crashes (returning
# None simply disables NTFF tracing).
try:
    import antenv.axon_hooks  # noqa: F401
except Exception:
    try:
        import antenv

        _m = types.ModuleType("antenv.axon_hooks")
        _HOOK = [None]

        def _get_hook():
            if _HOOK[0] is None:
                try:
                    from trn_agent_boot.trn_boot import _ntff_profile_via_ctypes

                    _HOOK[0] = _ntff_profile_via_ctypes("/opt/axon/libaxon_pjrt.so")
                except Exception:
                    _HOOK[0] = None
            return _HOOK[0]

        _m.get_axon_ntff_profile_hook = _get_hook
        _m.set_axon_ntff_profile_hook = lambda h: _HOOK.__setitem__(0, h)
        sys.modules["antenv.axon_hooks"] = _m
        antenv.axon_hooks = _m
    except Exception:
        pass

import concourse.bass as bass
import concourse.bass_utils as bu
import concourse.mybir as mybir
import concourse.tile as tile

# ---------------------------------------------------------------------------
# Shim 1: this container's walrus accepts only ONE sync-wait per instruction
# ("Too many sync wait commands"); Tile attaches several. Rewrite the BIR
# JSON so excess waits ride on same-engine NoOps immediately before the
# instruction (engine streams are in-order, so this is equivalent).
# Shim 2: upload_artifacts wants a cloud bucket; keep artifacts local.
# ---------------------------------------------------------------------------
_MAX_WAITS = 1


def _split_multi_waits(bir_bytes: bytes) -> bytes:
    d = json.loads(bir_bytes)
    ctr = 0
    changed = False
    for fn in d.get("functions", []):
        for bb in fn.get("blocks", []):
            out = []
            for inst in bb.get("instructions", []):
                si = inst.get("sync_info")
                waits = (si or {}).get("on_wait") or []
                if len(waits) > _MAX_WAITS:
                    changed = True
                    idx = 0
                    while len(waits) - idx > _MAX_WAITS:
                        chunk = waits[idx : idx + _MAX_WAITS]
                        idx += _MAX_WAITS
                        ctr += 1
                        nop = {
                            "engine": inst["engine"],
                            "ins": [],
                            "outs": [],
                            "name": f"I-wsplit-{ctr}",
                            "opcode": "NoOp",
                            "sync_info": {"on_update": [], "on_wait": chunk},
                        }
                        if "debug" in inst:
                            nop["debug"] = inst["debug"]
                        out.append(nop)
                    si["on_wait"] = waits[idx:]
                out.append(inst)
            bb["instructions"] = out
    return json.dumps(d).encode() if changed else bir_bytes


if not getattr(bass.Bass, "_wait_split_shim", False):
    _orig_to_json = bass.Bass.to_json_bytes

    def _to_json_bytes(self) -> bytes:
        return _split_multi_waits(_orig_to_json(self))

    bass.Bass.to_json_bytes = _to_json_bytes
    bass.Bass._wait_split_shim = True
    bu.upload_artifacts = lambda tmpdir: tmpdir

# ---------------------------------------------------------------------------
# Problem shapes (hardcoded per contract)
# ---------------------------------------------------------------------------
N, L, D = 4, 4096, 2048  # batches, seq len (q and k), model dim
H, P = 16, 128  # heads, head dim
NCORES = 8
HL = H // 2  # heads per core (head-group of 8)
OW = HL * P  # per-core projected width (1024)
DC = D // P  # 16 contraction chunks

F32 = mybir.dt.float32
F32R = mybir.dt.float32r
# The ACT tables in this walrus build ship no softplus; synthesize the
# numerically stable form softplus(x) = max(x,0) + ln(1 + exp(-|x|)) from
# set 6 ("natural_log_exp_and_others": abs/exp/ln in one resident table).
ABS = mybir.ActivationFunctionType.Abs
EXP = mybir.ActivationFunctionType.Exp
LN = mybir.ActivationFunctionType.Ln
MUL = mybir.AluOpType.mult
MAX = mybir.AluOpType.max
ADD = mybir.AluOpType.add

# kv_aug per-head column offsets inside the 3-bank PSUM accumulator:
# 3 heads per 2 KiB bank (129 fp32 columns each, none crossing a bank edge).
_KV_BASE = [(h // 3) * 512 + (h % 3) * 129 for h in range(HL)]

TRACE = False  # set True (e.g. from test.py) to capture NTFF profile
LAST_EXEC_TIME_NS = None

# Precision mode for all matmuls:
#   "f32r": projections in float32r (full rate, ~1.6e-4 end rel err),
#           attention stage fp32 (1/4-rate small matmuls, widened f32r num).
#   "fp16": everything fp16 inputs + fp32 PSUM accumulation (full rate,
#           ~11-bit input rounding => ~5e-4 end rel err, fastest).
#   "bf16": like fp16 but 8-bit mantissa (~2e-3 end rel err).
PREC = "fp16"

N_WARM = 110  # dummy matmuls to warm the HAM clock gate while DMAs land

_CACHED_NC = {}


def _build_nc(kc: int, qchunks: tuple) -> bass.Bass:
    """Build the per-core program for kc 128-key chunks and the given
    query chunk widths (each a multiple of 128; 512 in steady state)."""
    from contextlib import ExitStack

    qtot = sum(qchunks)
    qcb = qtot // P  # query keep-mask columns

    lp = {"fp16": mybir.dt.float16, "bf16": mybir.dt.bfloat16}.get(PREC)
    IN_DT = F32R if lp is None else lp  # projections (dram + sbuf operands)
    att_dt = F32 if lp is None else lp  # kv-stage operands (pk, v_aug)
    # num-stage matmul: fp16/bf16 run full-rate at any width; the f32r path
    # widens to 258 moving columns (>=256 streams at 1 cyc/row vs fp32's 4)
    # — the upper 129 columns are discarded junk.
    num_dt = F32R if lp is None else lp
    NW = 258 if lp is None else 129
    KV_W = HL * 129 + (129 if lp is None else 0)  # pad so h=7 reads 258 cols
    # fp16 output staging+DMA (host converts back): halves the output
    # traffic and the kernel-tail DMA; adds ~2.4e-4 relative rounding.
    OUT_DT = F32 if lp is None else mybir.dt.float16

    nc = bass.Bass()
    # qT/kT are host-packed: column block c holds that chunk's activations
    # with layout [p, dc*W + j] = act[chunk_off + j, dc*128 + p], so a chunk
    # loads as ONE dma_start with a contiguous per-partition run and the
    # dc-slices come out as plain column slices.
    qT = nc.dram_tensor("qT", (P, DC * qtot), IN_DT, kind="ExternalInput")
    kT = nc.dram_tensor("kT", (P, kc * D), IN_DT, kind="ExternalInput")
    # weights host-packed the same way ([p, dc*W + col] = W.T[dc*128+p, col])
    # with wk/wv interleaved per dc: each dma_start on the software DGE
    # costs ~0.65us of serial descriptor generation, so the critical opening
    # weight stream ships as 8 big 1MB pieces (8KB packets) in exactly the
    # order the matmuls consume them, instead of 32 row-tile DMAs.
    wq = nc.dram_tensor("wq", (P, DC * OW), IN_DT, kind="ExternalInput")
    wkv = nc.dram_tensor("wkv", (P, DC * 2 * OW), IN_DT, kind="ExternalInput")
    qm = nc.dram_tensor("qm", (P, qcb), F32, kind="ExternalInput")
    km = nc.dram_tensor("km", (P, kc), F32, kind="ExternalInput")
    out = nc.dram_tensor("out", (qtot, OW), OUT_DT, kind="ExternalOutput")

    with tile.TileContext(nc) as tc, ExitStack() as outer:
        # ------ PE warmup: the HAM clock gate starts at 1.2GHz and needs
        # ~3.4us of sustained activity to release to 2.4GHz. The opening
        # DMAs take ~10us to land; fill that window with dependency-free
        # matmuls so real work starts warm. Pool closes to free the bank.
        with (
            tc.tile_pool(name="warm", bufs=1) as warmp,
            tc.tile_pool(name="warmps", bufs=1, space="PSUM") as warmpsp,
        ):
            wt = warmp.tile([P, P], IN_DT, name="wt")
            nc.gpsimd.memset(wt[:], 0.0)
            wps = warmpsp.tile([P, P], F32, name="wps")
            for _ in range(N_WARM):
                nc.tensor.matmul(wps[:], wt[:], wt[:], start=True, stop=True)

        misc = outer.enter_context(tc.tile_pool(name="misc", bufs=1))
        kvpool = outer.enter_context(tc.tile_pool(name="kvsb", bufs=1))
        # qt pool lives at OUTER scope so its SBUF region is disjoint from
        # the phase-A pools: the first chunks' qt DMAs (ring-ahead on the
        # gpsimd queue) then stream in during phase A with no anti-deps.
        qtp = outer.enter_context(tc.tile_pool(name="qt", bufs=2))
        wqp = outer.enter_context(tc.tile_pool(name="wqe", bufs=1))
        qm_sb = misc.tile([P, qcb], F32)
        km_sb = misc.tile([P, kc], F32)
        nc.scalar.dma_start(qm_sb[:], qm[:])
        nc.scalar.dma_start(km_sb[:], km[:])
        kv_sb = kvpool.tile([P, KV_W], num_dt)

        # ------ Phase A: K/V projection + kv accumulation ------------------
        # One pass, all 8 heads. Each kt d-chunk slice serves 4 consecutive
        # matmuls (K and V, two 512-wide o-halves each).
        esA = ExitStack()
        wkvp = esA.enter_context(tc.tile_pool(name="wkv", bufs=1))
        ktp = esA.enter_context(tc.tile_pool(name="kt", bufs=3))
        pkp = esA.enter_context(tc.tile_pool(name="pk", bufs=3))
        vap = esA.enter_context(tc.tile_pool(name="vaug", bufs=3))
        pps = esA.enter_context(tc.tile_pool(name="projps", bufs=5, space="PSUM"))
        kvps = esA.enter_context(tc.tile_pool(name="kvps", bufs=1, space="PSUM"))
        kv_ps = kvps.tile([P, 1536], F32)

        # kt chunk 0 FIRST on the sync ring so the opening matmuls wait on
        # 0.5 MB; weights stream concurrently on the gpsimd + scalar rings.
        kt0 = ktp.tile([P, D], IN_DT, tag="kt", name="kt0")
        nc.sync.dma_start(kt0[:], kT[:, 0:D])

        wkv_sb = wkvp.tile([P, DC * 2 * OW], IN_DT, name="wkv_sb")
        NPC = 2 * 2 * OW  # dma piece = 2 dc groups (1 MB fp16)
        for j in range(DC * 2 * OW // NPC):
            nc.gpsimd.dma_start(
                wkv_sb[:, j * NPC : (j + 1) * NPC],
                wkv[:, j * NPC : (j + 1) * NPC],
            )

        def wk_slice(dc, half):
            base = dc * 2 * OW + half * 512
            return wkv_sb[:, base : base + 512]

        def wv_slice(dc, half):
            base = dc * 2 * OW + OW + half * 512
            return wkv_sb[:, base : base + 512]

        # wq's DMA is deferred until the opening chunks have consumed wk/wv:
        # the DMA engines are a shared ~358 GB/s pool, and letting the
        # (not-yet-needed) wq/qt stream run early starves the critical
        # phase-A weight loads.
        wq_sb = wqp.tile([P, DC * OW], IN_DT, name="wq_sb")

        bank_start = {}

        def emit_kv_mms(c, pk_sb, va_sb):
            for h in range(HL):
                bank_first = h % 3 == 0
                mm = nc.tensor.matmul(
                    kv_ps[:, _KV_BASE[h] : _KV_BASE[h] + 129],
                    pk_sb[:, h * P : (h + 1) * P],
                    va_sb[:, h * 129 : (h + 1) * 129],
                    start=(c == 0 and bank_first),
                    stop=(c == kc - 1),
                    skip_group_check=True,
                )
                if c == 0:
                    # start=True clears has_written for the whole PSUM bank;
                    # siblings must come after their bank's clear.
                    if bank_first:
                        bank_start[h // 3] = mm
                    else:
                        tile.add_dep_helper(
                            mm.ins,
                            bank_start[h // 3].ins,
                            reason="kv bank has_written clear order",
                        )

        # kv matmuls for chunk c are emitted after chunk c+1's projection
        # matmuls: their pk operand is only ready ~3us after chunk c's last
        # projection, so this keeps PE fed meanwhile.
        pending = None
        pps_allocs = 0
        wq_anchor = None
        wq_emitted = False
        for c in range(kc):
            if c == 0:
                kt_sb = kt0
            else:
                kt_sb = ktp.tile([P, D], IN_DT, tag="kt", name=f"kt{c}")
                nc.sync.dma_start(kt_sb[:], kT[:, c * D : (c + 1) * D])
            if not wq_emitted and wq_anchor is not None:
                # release the deferred wq load
                dma = nc.scalar.dma_start(wq_sb[:], wq[:])
                tile.add_dep_helper(
                    dma.ins,
                    wq_anchor.ins,
                    sync=True,
                    reason="defer wq behind wk/wv",
                )
                wq_emitted = True
            if c == kc - 1:
                # rotate the proj-PSUM allocation so the final chunk's tiles
                # land on slots 1..4 and slot 0 (which phase B's first qp
                # tile will alias) was last touched two chunks earlier —
                # phase B's opening matmul then has no WAR stall.
                for _ in range((1 - pps_allocs) % 5):
                    pps.tile([P, 512], F32, tag="proj", name="spacer")
                    pps_allocs += 1
            kp0 = pps.tile([P, 512], F32, tag="proj", name="kp0")
            kp1 = pps.tile([P, 512], F32, tag="proj", name="kp1")
            vp0 = pps.tile([P, 512], F32, tag="proj", name="vp0")
            vp1 = pps.tile([P, 512], F32, tag="proj", name="vp1")
            pps_allocs += 4
            for dc in range(DC):
                lhsT = kt_sb[:, dc * P : (dc + 1) * P]
                st = dict(start=(dc == 0), stop=(dc == DC - 1))
                nc.tensor.matmul(kp0[:], lhsT, wk_slice(dc, 0), **st)
                nc.tensor.matmul(kp1[:], lhsT, wk_slice(dc, 1), **st)
                nc.tensor.matmul(vp0[:], lhsT, wv_slice(dc, 0), **st)
                mmv = nc.tensor.matmul(vp1[:], lhsT, wv_slice(dc, 1), **st)
                if c == min(1, kc - 1) and dc == DC - 1:
                    wq_anchor = mmv

            if pending is not None:
                emit_kv_mms(*pending)

            pk_sb = pkp.tile([P, OW], att_dt, tag="pk", name="pk")
            for half, kp in ((0, kp0), (1, kp1)):
                sa = pkp.tile([P, 512], F32, tag="sa", name="sa")
                sb = pkp.tile([P, 512], F32, tag="sb", name="sb")
                nc.scalar.activation(sa[:], kp[:], ABS)
                nc.scalar.activation(sb[:], sa[:], EXP, scale=-1.0)
                nc.scalar.activation(sa[:], sb[:], LN, bias=1.0)
                nc.vector.scalar_tensor_tensor(
                    pk_sb[:, half * 512 : (half + 1) * 512],
                    kp[:],
                    0.0,
                    sa[:],
                    MAX,
                    ADD,
                )
            nc.vector.tensor_scalar_mul(pk_sb[:], pk_sb[:], km_sb[:, c : c + 1])

            va_sb = vap.tile([P, HL * 129], att_dt, tag="vaug", name="va")
            nc.vector.memset(
                va_sb[:].rearrange("p (h x) -> p h x", x=129)[:, :, 128:129], 1.0
            )
            for h in range(HL):
                src = vp0 if h < 4 else vp1
                off = (h % 4) * P
                nc.vector.tensor_copy(
                    va_sb[:, h * 129 : h * 129 + P], src[:, off : off + P]
                )
            pending = (c, pk_sb, va_sb)

        if not wq_emitted:
            dma = nc.scalar.dma_start(wq_sb[:], wq[:])
            tile.add_dep_helper(
                dma.ins, wq_anchor.ins, sync=True, reason="defer wq behind wk/wv"
            )

        # Dependency-free bridge matmuls: the final chunk's kv matmuls wait
        # ~2.5us for its softplus, and the in-order PE queue would idle.
        # These land on proj-PSUM slot 0 (free since two chunks ago, thanks
        # to the spacer rotation) and read the resident kt tile, so they
        # issue immediately and keep the PE busy+warm across the boundary.
        bridge = pps.tile([P, 512], F32, tag="proj", name="bridge")
        pps_allocs += 1
        for _ in range(14):
            nc.tensor.matmul(
                bridge[:], kt_sb[:, 0:P], kt_sb[:, 0:512], start=True, stop=True
            )
        emit_kv_mms(*pending)

        for h in range(HL):
            nc.vector.tensor_copy(
                kv_sb[:, h * 129 : (h + 1) * 129],
                kv_ps[:, _KV_BASE[h] : _KV_BASE[h] + 129],
            )
        if KV_W > HL * 129:
            # f32r-typed zero pad (junk columns read by head 7's widened MM);
            # written by a DVE op so the FP32R-rounding verifier is satisfied.
            nc.vector.tensor_scalar_mul(
                kv_sb[:, HL * 129 : KV_W], kv_sb[:, 0:129], 0.0
            )
        esA.close()

        # ------ Phase B: Q projection + attention epilogue -----------------
        with (
            tc.tile_pool(name="pq", bufs=4) as pqp,
            tc.tile_pool(name="sc", bufs=4) as scp,
            tc.tile_pool(name="st", bufs=3) as stp,
            tc.tile_pool(name="qpps", bufs=4, space="PSUM") as qpps,
            tc.tile_pool(name="nmps", bufs=4, space="PSUM") as nmps,
        ):
            st_tiles = {}

            def emit_num(ci, o, W, h, pq_sb):
                # results stage into st (partition=l%128, cols j*OW+o) and
                # ship as ONE multi-run DMA per chunk — per-(h,j) output
                # DMAs cost ~0.6us of descriptor generation each.
                nj = W // P
                if h == 0:
                    st_tiles[ci] = stp.tile(
                        [P, nj * OW], OUT_DT, tag="st", name="st"
                    )
                st = st_tiles[ci]
                for j in range(nj):
                    nm = nmps.tile([P, NW], F32, tag="nm", name="nm")
                    nc.tensor.matmul(
                        nm[:],
                        pq_sb[:, j * P : (j + 1) * P],
                        kv_sb[:, h * 129 : h * 129 + NW],
                        start=True,
                        stop=True,
                    )
                    sc = scp.tile([P, 1], F32, tag="sc", name="sc")
                    nc.vector.reciprocal(sc[:], nm[:, 128:129])
                    col = o // P + j
                    nc.vector.tensor_scalar(
                        st[:, j * OW + h * P : j * OW + (h + 1) * P],
                        nm[:, 0:P],
                        sc[:, 0:1],
                        qm_sb[:, col : col + 1],
                        MUL,
                        MUL,
                    )
                if h == HL - 1:
                    nc.sync.dma_start(
                        out[o : o + W, :].rearrange("(j p) o -> p j o", p=P),
                        st[:].rearrange("p (j o) -> p j o", o=OW),
                    )
                    del st_tiles[ci]

            # num matmuls for step (ci,h) are emitted two steps behind the
            # projection matmuls (pq is ~2.5us of ACT/DVE behind qp; depth-2
            # keeps the PE fed through the boundary where the ACT queue is
            # still draining phase A's last softplus).
            from collections import deque

            pendq = deque()
            o = 0
            for ci, W in enumerate(qchunks):
                qt_sb = qtp.tile([P, DC * W], IN_DT, tag="qt", name=f"qt{ci}")
                nc.gpsimd.dma_start(
                    qt_sb[:], qT[:, DC * o : DC * (o + W)]
                )
                for h in range(HL):
                    qp = qpps.tile([P, W], F32, tag="qp", name="qp")
                    for dc in range(DC):
                        nc.tensor.matmul(
                            qp[:],
                            wq_sb[:, dc * OW + h * P : dc * OW + (h + 1) * P],
                            qt_sb[:, dc * W : (dc + 1) * W],
                            start=(dc == 0),
                            stop=(dc == DC - 1),
                        )
                    # depth-2 through the boundary; depth-1 on the last chunk
                    # so the kernel tail (drain with no projs left) is short
                    depth = 1 if ci == len(qchunks) - 1 else 2
                    while len(pendq) >= depth:
                        emit_num(*pendq.popleft())
                    pq_sb = pqp.tile([P, W], num_dt, tag="pq", name="pq")
                    sa = pqp.tile([P, W], F32, tag="sqa", name="sqa")
                    sb = pqp.tile([P, W], F32, tag="sqb", name="sqb")
                    nc.scalar.activation(sa[:], qp[:], ABS)
                    nc.scalar.activation(sb[:], sa[:], EXP, scale=-1.0)
                    nc.scalar.activation(sa[:], sb[:], LN, bias=1.0)
                    nc.vector.scalar_tensor_tensor(
                        pq_sb[:], qp[:], 0.0, sa[:], MAX, ADD
                    )
                    pendq.append((ci, o, W, h, pq_sb))
                o += W
            while pendq:
                emit_num(*pendq.popleft())
    return nc


def _get_nc(kc: int, qchunks: tuple) -> bass.Bass:
    key = (PREC, kc, qchunks)
    if key not in _CACHED_NC:
        _CACHED_NC[key] = _build_nc(kc, qchunks)
    return _CACHED_NC[key]


def _qchunk_widths(qtot: int) -> tuple:
    """Split qtot (a multiple of 128) into 512-wide chunks plus at most one
    smaller remainder chunk, remainder LAST so the kernel tail is short."""
    nfull, rem = divmod(qtot, 512)
    w = [512] * nfull
    if rem:
        w.append(rem)
    return tuple(w)


def kernel(query, key, Wq, Wk, Wv, query_padding_mask, key_padding_mask):
    global LAST_EXEC_TIME_NS
    query = np.asarray(query, dtype=np.float32)
    key = np.asarray(key, dtype=np.float32)
    Wq = np.asarray(Wq, dtype=np.float32)
    Wk = np.asarray(Wk, dtype=np.float32)
    Wv = np.asarray(Wv, dtype=np.float32)
    qmask = np.asarray(query_padding_mask)
    kmask = np.asarray(key_padding_mask)

    in_dt = np.float32
    if PREC == "fp16":
        in_dt = np.float16
    elif PREC == "bf16":
        import ml_dtypes

        in_dt = ml_dtypes.bfloat16

    # Compaction: gather unpadded rows, pad to a common (over batches)
    # multiple of 128. Padded rows are zeros with keep=0 so they contribute
    # exactly nothing; query rows are scattered back on host.
    kidxs = [np.flatnonzero(~kmask[n]) for n in range(N)]
    qidxs = [np.flatnonzero(~qmask[n]) for n in range(N)]
    kmax = max(max(len(ix) for ix in kidxs), 1)
    qmax = max(max(len(ix) for ix in qidxs), 1)
    kc = -(-kmax // P)  # key chunks of 128
    ktot = kc * P
    qtot = -(-qmax // P) * P
    qchunks = _qchunk_widths(qtot)

    nc = _get_nc(kc, qchunks)

    in_maps = []
    for c in range(NCORES):
        n, g = c // 2, c % 2
        sl = slice(g * OW, (g + 1) * OW)
        kidx, qidx = kidxs[n], qidxs[n]

        key_c = np.zeros((ktot, D), np.float32)
        key_c[: len(kidx)] = key[n][kidx]
        # kT packing: [p, c*D + dc*128 + j] = key_c[c*128 + j, dc*128 + p]
        kT2 = np.ascontiguousarray(
            key_c.reshape(kc, P, DC, P).transpose(3, 0, 2, 1).reshape(P, kc * D)
        ).astype(in_dt)
        km2 = np.zeros(ktot, np.float32)
        km2[: len(kidx)] = 1.0
        km2 = np.ascontiguousarray(km2.reshape(kc, P).T)

        query_c = np.zeros((qtot, D), np.float32)
        query_c[: len(qidx)] = query[n][qidx]
        # qT packing per chunk: [p, dc*W + j] = query_c[o + j, dc*128 + p]
        blocks = []
        o = 0
        for W in qchunks:
            blocks.append(
                query_c[o : o + W].reshape(W, DC, P).transpose(2, 1, 0).reshape(P, DC * W)
            )
            o += W
        qT2 = np.ascontiguousarray(np.concatenate(blocks, axis=1)).astype(in_dt)
        qm2 = np.zeros(qtot, np.float32)
        qm2[: len(qidx)] = 1.0
        qm2 = np.ascontiguousarray(qm2.reshape(-1, P).T)

        # weights packed as [p, dc*W + col] = W[sl].T[dc*128+p, col], with
        # wk/wv interleaved per dc so one linear DMA stream arrives in
        # exactly the order phase A consumes it
        ak = Wk[sl, :].T.reshape(DC, P, OW)
        av = Wv[sl, :].T.reshape(DC, P, OW)
        wkv2 = np.ascontiguousarray(
            np.concatenate([ak, av], axis=2).transpose(1, 0, 2).reshape(P, DC * 2 * OW)
        ).astype(in_dt)
        wq2 = np.ascontiguousarray(
            Wq[sl, :].T.reshape(DC, P, OW).transpose(1, 0, 2).reshape(P, DC * OW)
        ).astype(in_dt)

        in_maps.append(
            {
                "qT": qT2,
                "kT": kT2,
                "wq": wq2,
                "wkv": wkv2,
                "qm": qm2,
                "km": km2,
            }
        )

    res = bu.run_bass_kernel_spmd(
        nc, in_maps, core_ids=list(range(NCORES)), trace=TRACE
    )
    LAST_EXEC_TIME_NS = res.exec_time_ns

    full = np.zeros((N, L, D), dtype=np.float32)
    for c in range(NCORES):
        n, g = c // 2, c % 2
        qidx = qidxs[n]
        full[n, qidx, g * OW : (g + 1) * OW] = res.results[c]["out"][: len(qidx)]
    return full
